# revision 1
# baseline (speedup 1.0000x reference)
"""DeepseekV3 MLA attention kernel for 8 Trainium2 NeuronCores.

Sharding: 2-way data-parallel over batch x 4-way tensor-parallel over heads.
Core c handles batch b = c // 4 and heads [4*(c%4) .. 4*(c%4)+4).

Per core:
  phase 1: qT = (hidden @ (w_q_a @ w_q_b_g)).T  -- LoRA A*B fused on host
           (RoPE applied on the rope rows), kv_c = hidden @ w_kv_a
  phase 2: kT/v head projections from kv_c (RoPE on k rope rows)
  phase 3: causal attention (scoresT layout, max-free softmax with
           ones-matmul denominators) + this head-group's slice of w_o.
Host sums the 4 partial outputs per batch.

All matmuls run as float32r (fp32 storage, fast PE mode).
"""

from contextlib import ExitStack
from dataclasses import dataclass

import numpy as np

import concourse.bacc as bacc
import concourse.mybir as mybir
import concourse.tile as tile

F32 = mybir.dt.float32
F32R = mybir.dt.float32r


@dataclass(frozen=True)
class Cfg:
    S: int = 2048          # sequence length (per batch)
    HID: int = 2048        # hidden dim
    QLR: int = 1536        # q lora rank (host-side only)
    KVLR: int = 512        # kv lora rank
    NH_G: int = 4          # heads per core
    DN: int = 128          # nope dim
    DR: int = 64           # rope dim
    DV: int = 128          # v head dim
    S1T: int = 512         # phase-1 s-block width
    S2T: int = 512         # phase-2 s-tile width
    QT: int = 512          # attention q-tile width

    @property
    def QFN(self):
        return self.NH_G * self.DN      # fused q nope cols

    @property
    def QFR(self):
        return (self.NH_G // 2) * 128   # fused q rope cols (pair-packed)

    @property
    def SCALE(self):
        return 1.0 / float(np.sqrt(self.DN + self.DR))


CFG = Cfg()


def build_nc(C: Cfg, reps: int = 1):
    nc = bacc.Bacc("TRN2", target_bir_lowering=False, debug=False, num_devices=8)
    P = 128
    HO = C.HID // P
    NS1 = C.S // C.S1T
    NS2 = C.S // C.S2T
    KVC = C.KVLR // P
    NPAIR = C.NH_G // 2
    NQT = C.S // C.QT
    NDIAG = C.QT // P
    NVS = C.S // P
    NQN = C.QFN // P
    NOT = C.HID // 512

    # ---- kernel I/O ----
    hT = nc.dram_tensor("hT", [C.HID, C.S], F32R, kind="ExternalInput").ap()
    w_qf = nc.dram_tensor("w_qf", [C.HID, C.QFN + C.QFR], F32R, kind="ExternalInput").ap()
    w_kva = nc.dram_tensor("w_kva", [C.HID, C.KVLR], F32R, kind="ExternalInput").ap()
    w_kbn = nc.dram_tensor("w_kbn", [C.KVLR, C.NH_G * C.DN], F32R, kind="ExternalInput").ap()
    w_kbr = nc.dram_tensor("w_kbr", [C.KVLR, C.NH_G * C.DR], F32R, kind="ExternalInput").ap()
    w_vb = nc.dram_tensor("w_vb", [C.KVLR, C.NH_G * C.DV], F32R, kind="ExternalInput").ap()
    w_ob = nc.dram_tensor("w_ob", [C.NH_G * C.DV, C.HID], F32R, kind="ExternalInput").ap()
    cos2 = nc.dram_tensor("cos2", [P, C.S], F32, kind="ExternalInput").ap()
    ssin2 = nc.dram_tensor("ssin2", [P, C.S], F32, kind="ExternalInput").ap()
    dmask = nc.dram_tensor("dmask", [C.QT, C.QT], F32, kind="ExternalInput").ap()
    outp = nc.dram_tensor("outp", [C.S, C.HID], F32, kind="ExternalOutput").ap()

    # ---- DRAM scratch ----
    kv_d = nc.dram_tensor("kv_scr", [C.KVLR, C.S], F32R).ap()
    qTn_d = nc.dram_tensor("qTn_scr", [C.QFN, C.S], F32R).ap()
    qTr_d = nc.dram_tensor("qTr_scr", [C.QFR, C.S], F32R).ap()

    with tile.TileContext(nc) as tc:
        for rep in range(reps):
            with ExitStack() as tctx:
                tab_pool = tctx.enter_context(tc.tile_pool(name=f"tab{rep}", bufs=1))
                cos_sb = tab_pool.tile([P, C.S], F32)
                ssin_sb = tab_pool.tile([P, C.S], F32)
                nc.sync.dma_start(out=cos_sb[:], in_=cos2)
                nc.sync.dma_start(out=ssin_sb[:], in_=ssin2)

                def rope_evict(rp_pool, ps_nat, ps_sw, dst_ap, s0, slen):
                    """psum of rope rows -> roped into dst (DMA partition swap)."""
                    tmp = rp_pool.tile([P, slen], F32R, tag="rope_tmp")
                    nc.vector.tensor_copy(tmp[:], ps_nat[:])
                    qs = rp_pool.tile([P, slen], F32R, tag="rope_qs")
                    for g in range(4):
                        nc.sync.dma_start(
                            out=qs[(g ^ 1) * 32:(g ^ 1) * 32 + 32, :],
                            in_=tmp[g * 32:(g + 1) * 32, :])
                    m1 = rp_pool.tile([P, slen], F32R, tag="rope_m1")
                    nc.vector.tensor_mul(m1[:], tmp[:], cos_sb[:, s0:s0 + slen])
                    nc.vector.tensor_mul(qs[:], qs[:], ssin_sb[:, s0:s0 + slen])
                    nc.vector.tensor_add(dst_ap, m1[:], qs[:])

                # ===== Phase 1: fused q projection + kv latent, from hidden =====
                with ExitStack() as ctx:
                    wq_pool = ctx.enter_context(tc.tile_pool(name=f"wqf{rep}", bufs=1))
                    ht_pool = ctx.enter_context(tc.tile_pool(name=f"ht{rep}", bufs=2))
                    ev_pool = ctx.enter_context(tc.tile_pool(name=f"s1ev{rep}", bufs=4))
                    rp_pool = ctx.enter_context(tc.tile_pool(name=f"s1rp{rep}", bufs=3))
                    ps_pool = ctx.enter_context(
                        tc.tile_pool(name=f"s1ps{rep}", bufs=6, space="PSUM"))

                    wqf_sb = wq_pool.tile([P, HO, C.QFN + C.QFR], F32R)
                    wkva_sb = wq_pool.tile([P, HO, C.KVLR], F32R)
                    wqf_r = w_qf.rearrange("(ho hi) c -> hi ho c", hi=P)
                    wkva_r = w_kva.rearrange("(ho hi) c -> hi ho c", hi=P)
                    for ho in range(HO):
                        nc.sync.dma_start(
                            out=wqf_sb[:, ho, :], in_=wqf_r[:, ho, :])
                        nc.sync.dma_start(
                            out=wkva_sb[:, ho, :], in_=wkva_r[:, ho, :])
                    hT_r = hT.rearrange("(ho hi) s -> hi ho s", hi=P)
                    for st in range(NS1):
                        s0 = st * C.S1T
                        ht_sb = ht_pool.tile([P, HO, C.S1T], F32R)
                        nc.scalar.dma_start(out=ht_sb[:], in_=hT_r[:, :, s0:s0 + C.S1T])

                        def accum(lhs_sb, col0):
                            ps = ps_pool.tile([P, C.S1T], F32, tag="ps1")
                            for h in range(HO):
                                nc.tensor.matmul(
                                    ps[:], lhs_sb[:, h, col0:col0 + P],
                                    ht_sb[:, h, :],
                                    start=(h == 0), stop=(h == HO - 1))
                            return ps

                        for t in range(NQN):
                            ps = accum(wqf_sb, t * P)
                            ev = ev_pool.tile([P, C.S1T], F32R)
                            nc.vector.tensor_copy(ev[:], ps[:])
                            nc.sync.dma_start(
                                out=qTn_d[t * P:(t + 1) * P, s0:s0 + C.S1T], in_=ev[:])
                        for pr in range(NPAIR):
                            ps_nat = accum(wqf_sb, C.QFN + pr * P)
                            ev = ev_pool.tile([P, C.S1T], F32R)
                            rope_evict(rp_pool, ps_nat, None, ev[:], s0, C.S1T)
                            nc.sync.dma_start(
                                out=qTr_d[pr * P:(pr + 1) * P, s0:s0 + C.S1T], in_=ev[:])
                        for cc in range(KVC):
                            ps = accum(wkva_sb, cc * P)
                            ev = ev_pool.tile([P, C.S1T], F32R)
                            nc.vector.tensor_copy(ev[:], ps[:])
                            nc.sync.dma_start(
                                out=kv_d[cc * P:(cc + 1) * P, s0:s0 + C.S1T], in_=ev[:])

                # ===== Phase 2: kT / v projections (resident for attention) =====
                per_pool = tctx.enter_context(tc.tile_pool(name=f"persist{rep}", bufs=1))
                kTn_sb = per_pool.tile([P, C.NH_G, C.S], F32R)
                kTr_sb = per_pool.tile([P, NPAIR, C.S], F32R)
                v_sb = per_pool.tile([P, NVS, C.NH_G * C.DV], F32R)
                with ExitStack() as ctx:
                    wk_pool = ctx.enter_context(tc.tile_pool(name=f"wk{rep}", bufs=1))
                    kv_pool = ctx.enter_context(tc.tile_pool(name=f"kvs{rep}", bufs=2))
                    rp_pool = ctx.enter_context(tc.tile_pool(name=f"s2rp{rep}", bufs=3))
                    ps_pool = ctx.enter_context(
                        tc.tile_pool(name=f"s2ps{rep}", bufs=4, space="PSUM"))

                    wkn_sb = wk_pool.tile([P, KVC, C.NH_G * C.DN], F32R)
                    wkr_sb = wk_pool.tile([P, KVC, C.NH_G * C.DR], F32R)
                    wv_sb = wk_pool.tile([P, KVC, C.NH_G * C.DV], F32R)
                    nc.sync.dma_start(
                        out=wkn_sb[:], in_=w_kbn.rearrange("(co ci) m -> ci co m", ci=P))
                    nc.sync.dma_start(
                        out=wkr_sb[:], in_=w_kbr.rearrange("(co ci) m -> ci co m", ci=P))
                    nc.sync.dma_start(
                        out=wv_sb[:], in_=w_vb.rearrange("(co ci) m -> ci co m", ci=P))
                    kv_r = kv_d.rearrange("(co ci) s -> ci co s", ci=P)
                    for st in range(NS2):
                        s0 = st * C.S2T
                        kv_sb = kv_pool.tile([P, KVC, C.S2T], F32R)
                        nc.sync.dma_start(out=kv_sb[:], in_=kv_r[:, :, s0:s0 + C.S2T])
                        for h in range(C.NH_G):
                            ps = ps_pool.tile([P, C.S2T], F32, tag="ps2")
                            for cch in range(KVC):
                                nc.tensor.matmul(
                                    ps[:], wkn_sb[:, cch, h * C.DN:(h + 1) * C.DN],
                                    kv_sb[:, cch, :],
                                    start=(cch == 0), stop=(cch == KVC - 1))
                            nc.vector.tensor_copy(kTn_sb[:, h, s0:s0 + C.S2T], ps[:])
                        for pr in range(NPAIR):
                            ps_nat = ps_pool.tile([P, C.S2T], F32, tag="ps2")
                            for cch in range(KVC):
                                nc.tensor.matmul(
                                    ps_nat[:], wkr_sb[:, cch, pr * P:(pr + 1) * P],
                                    kv_sb[:, cch, :],
                                    start=(cch == 0), stop=(cch == KVC - 1))
                            rope_evict(rp_pool, ps_nat, None,
                                       kTr_sb[:, pr, s0:s0 + C.S2T], s0, C.S2T)
                        for ssub in range(C.S2T // P):
                            vs = (s0 + ssub * P) // P
                            ps = ps_pool.tile([P, C.NH_G * C.DV], F32, tag="ps2")
                            for cch in range(KVC):
                                nc.tensor.matmul(
                                    ps[:], kv_sb[:, cch, ssub * P:(ssub + 1) * P],
                                    wv_sb[:, cch, :],
                                    start=(cch == 0), stop=(cch == KVC - 1))
                            nc.scalar.copy(v_sb[:, vs, :], ps[:])

                # ================= Phase 3: attention + w_o =================
                with ExitStack() as ctx:
                    const_pool = ctx.enter_context(tc.tile_pool(name=f"ac{rep}", bufs=1))
                    qn_pool = ctx.enter_context(tc.tile_pool(name=f"aqn{rep}", bufs=2))
                    qr_pool = ctx.enter_context(tc.tile_pool(name=f"aqr{rep}", bufs=2))
                    e_pool = ctx.enter_context(tc.tile_pool(name=f"ae{rep}", bufs=5))
                    d_pool = ctx.enter_context(tc.tile_pool(name=f"ad{rep}", bufs=2))
                    ao_pool = ctx.enter_context(tc.tile_pool(name=f"aao{rep}", bufs=2))
                    oev_pool = ctx.enter_context(tc.tile_pool(name=f"aoe{rep}", bufs=4))
                    ps_s = ctx.enter_context(
                        tc.tile_pool(name=f"apss{rep}", bufs=3, space="PSUM"))
                    ps_d = ctx.enter_context(
                        tc.tile_pool(name=f"apsd{rep}", bufs=1, space="PSUM"))
                    ps_o = ctx.enter_context(
                        tc.tile_pool(name=f"apso{rep}", bufs=2, space="PSUM"))
                    ps_w = ctx.enter_context(
                        tc.tile_pool(name=f"apsw{rep}", bufs=2, space="PSUM"))

                    ones_f = const_pool.tile([P, P], F32)
                    ones_sb = const_pool.tile([P, P], F32R)
                    nc.vector.memset(ones_f[:], 1.0)
                    nc.vector.tensor_copy(ones_sb[:], ones_f[:])
                    dm_sb = const_pool.tile([P, NDIAG, C.QT], F32)
                    nc.sync.dma_start(
                        out=dm_sb[:], in_=dmask.rearrange("(j ki) q -> ki j q", ki=P))
                    wo_sb = const_pool.tile([P, C.NH_G, C.HID], F32R)
                    nc.sync.dma_start(
                        out=wo_sb[:], in_=w_ob.rearrange("(h d) o -> d h o", d=P))

                    for qt in range(NQT):
                        q0 = qt * C.QT
                        nkt = (qt + 1) * C.QT // P
                        ao_sb = ao_pool.tile([P, C.NH_G, C.QT], F32R, tag="ao")
                        for pr in range(NPAIR):
                            qr_sb = qr_pool.tile([P, C.QT], F32R, tag="qr")
                            nc.sync.dma_start(
                                out=qr_sb[:],
                                in_=qTr_d[pr * P:(pr + 1) * P, q0:q0 + C.QT])
                            for hh in range(2):
                                h = pr * 2 + hh
                                qn_sb = qn_pool.tile([P, C.QT], F32R, tag="qn")
                                nc.sync.dma_start(
                                    out=qn_sb[:],
                                    in_=qTn_d[h * C.DN:(h + 1) * C.DN, q0:q0 + C.QT])
                                rsl = slice(hh * C.DR, (hh + 1) * C.DR)
                                psd = ps_d.tile([P, C.QT], F32, tag="psd")
                                pso = ps_o.tile([P, C.QT], F32, tag="pso")

                                def consume(e_prev, ktp):
                                    nc.tensor.matmul(
                                        psd[:], ones_sb[:], e_prev[:],
                                        start=(ktp == 0), stop=(ktp == nkt - 1))
                                    nc.tensor.matmul(
                                        pso[:],
                                        v_sb[:, ktp, h * C.DV:(h + 1) * C.DV],
                                        e_prev[:],
                                        start=(ktp == 0), stop=(ktp == nkt - 1))

                                pend = None
                                for kt in range(nkt):
                                    k0 = kt * P
                                    pss = ps_s.tile([P, C.QT], F32, tag="pss")
                                    nc.tensor.matmul(
                                        pss[:], kTn_sb[:, h, k0:k0 + P], qn_sb[:],
                                        start=True, stop=False)
                                    nc.tensor.matmul(
                                        pss[:], kTr_sb[rsl, pr, k0:k0 + P],
                                        qr_sb[rsl, :],
                                        start=False, stop=True)
                                    e_sb = e_pool.tile([P, C.QT], F32R, tag="e")
                                    nc.scalar.activation(
                                        e_sb[:], pss[:],
                                        mybir.ActivationFunctionType.Exp,
                                        scale=C.SCALE)
                                    j = kt - qt * NDIAG
                                    if j >= 0:
                                        nc.vector.tensor_mul(
                                            e_sb[:], e_sb[:], dm_sb[:, j, :])
                                    if pend is not None:
                                        consume(*pend)
                                    pend = (e_sb, kt)
                                consume(*pend)
                                rec = d_pool.tile([P, C.QT], F32, tag="rec")
                                nc.vector.reciprocal(rec[:], psd[:])
                                nc.vector.tensor_mul(ao_sb[:, h, :], pso[:], rec[:])
                        for qs in range(C.QT // P):
                            for ot in range(NOT):
                                psw = ps_w.tile([P, 512], F32, tag="psw")
                                for h in range(C.NH_G):
                                    nc.tensor.matmul(
                                        psw[:], ao_sb[:, h, qs * P:(qs + 1) * P],
                                        wo_sb[:, h, ot * 512:(ot + 1) * 512],
                                        start=(h == 0), stop=(h == C.NH_G - 1))
                                oev = oev_pool.tile([P, 512], F32)
                                nc.vector.tensor_copy(oev[:], psw[:])
                                nc.sync.dma_start(
                                    out=outp[q0 + qs * P:q0 + (qs + 1) * P,
                                             ot * 512:(ot + 1) * 512],
                                    in_=oev[:])

    nc.compile()
    return nc


def rope_tables(C: Cfg):
    """cos2/ssin2 [128, S]: two stacked 64-row blocks (head pairs share)."""
    inv = 1.0 / (10000.0 ** (np.arange(0, C.DR, 2, dtype=np.float64) / C.DR))
    freqs = np.arange(C.S, dtype=np.float64)[:, None] * inv[None, :]  # [S, 32]
    emb = np.concatenate([freqs, freqs], axis=1)  # [S, 64]
    cos = np.cos(emb).T.astype(np.float32)   # [64, S]
    sin = np.sin(emb).T.astype(np.float32)
    ssin = sin.copy()
    ssin[: C.DR // 2] = -ssin[: C.DR // 2]
    cos2 = np.concatenate([cos, cos], axis=0)     # [128, S]
    ssin2 = np.concatenate([ssin, ssin], axis=0)
    return np.ascontiguousarray(cos2), np.ascontiguousarray(ssin2)


def host_inputs(C: Cfg, inputs: dict, core: int):
    """Build the per-core input map from full inputs."""
    NH = inputs["w_q_nope"].shape[1] // C.DN
    groups = NH // C.NH_G
    b = core // groups
    g = core % groups
    hs = slice(g * C.NH_G, (g + 1) * C.NH_G)

    f32 = lambda x: np.ascontiguousarray(np.asarray(x, dtype=np.float32))

    def swap32(w):
        # swap the two 32-col halves of every 64-col group (rotate_half source)
        n = w.shape[1]
        return w.reshape(w.shape[0], n // 64, 2, 32)[:, :, ::-1, :].reshape(w.shape[0], n)

    hT = f32(inputs["hidden_states"][b].T)
    w_q_a = np.asarray(inputs["w_q_a"], dtype=np.float32)
    w_qbn = f32(inputs["w_q_nope"].reshape(C.QLR, NH, C.DN)[:, hs].reshape(C.QLR, -1))
    w_qbr = f32(inputs["w_q_rope"].reshape(C.QLR, NH, C.DR)[:, hs].reshape(C.QLR, -1))
    w_qfr = w_q_a @ w_qbr
    w_qf = f32(np.concatenate([w_q_a @ w_qbn, w_qfr], axis=1))
    w_kva = f32(inputs["w_kv_a"])
    w_kbn = f32(inputs["w_k_nope"].reshape(C.KVLR, NH, C.DN)[:, hs].reshape(C.KVLR, -1))
    w_kbr = f32(inputs["w_k_rope"].reshape(C.KVLR, NH, C.DR)[:, hs].reshape(C.KVLR, -1))
    w_vb = f32(inputs["w_v"].reshape(C.KVLR, NH, C.DV)[:, hs].reshape(C.KVLR, -1))
    w_ob = f32(inputs["w_o"].reshape(NH, C.DV, C.HID)[hs].reshape(-1, C.HID))
    cos2, ssin2 = rope_tables(C)
    cm = np.asarray(inputs["causal_mask"])[0, 0]
    dmask = np.ascontiguousarray(cm[-C.QT:, -C.QT:].T.astype(np.float32))
    return {
        "hT": hT, "w_qf": w_qf, "w_kva": w_kva,
        "w_kbn": w_kbn, "w_kbr": w_kbr, "w_vb": w_vb, "w_ob": w_ob,
        "cos2": cos2, "ssin2": ssin2, "dmask": dmask,
    }


_NC_CACHE = {}


def kernel(**inputs) -> np.ndarray:
    from concourse.bass_utils import run_bass_kernel_spmd

    C = CFG
    if "nc" not in _NC_CACHE:
        _NC_CACHE["nc"] = build_nc(C)
    nc = _NC_CACHE["nc"]

    in_maps = [host_inputs(C, inputs, c) for c in range(8)]
    res = run_bass_kernel_spmd(nc, in_maps, core_ids=list(range(8)))

    B = inputs["hidden_states"].shape[0]
    groups = 8 // B
    out = np.zeros((B, C.S, C.HID), dtype=np.float32)
    for c in range(8):
        out[c // groups] += res.results[c]["outp"]
    return out



# revision 2
# speedup vs baseline: 1.6943x; 1.6943x over previous
"""DeepseekV3 MLA attention kernel for 8 Trainium2 NeuronCores — v2.

Sharding: 2-way data-parallel over batch x 4-way tensor-parallel over heads.
Core c handles batch b = c // 4 and heads [4*(c%4) .. 4*(c%4)+4).

v2 design vs baseline:
  - all projection / AV / w_o matmuls in bf16 (same PE rate as fp32r but
    half the SBUF/DMA traffic, FWL weight loads, 2x DVE elementwise)
  - score matmuls in fp8e4 with DoubleRow perf mode: nope(128) + rope(64,
    zero-padded) packed as a 2-plane 256-deep contraction -> one matmul
    at 0.5 cycles/row
  - softmax denominators via DVE running-sum of e-tiles + one ones-matmul
    per (head, q-tile) instead of a ones-matmul per (head, k-tile)
  - causal diag shrink: score/exp/AV restricted to the unmasked column
    range on diagonal tiles
  - all intermediates (q, k, v) SBUF-resident, no DRAM scratch
  - phases software-pipelined per 512-wide s-tile:
    p1(st) -> p2(st) -> attention(qt=st)

All tolerances validated numerically on the CPU reference: bf16-everything
gives 4.3e-3 scale-relative max error; fp8 q/k adds ~6.7e-3 (tolerance 2e-2).
"""

from contextlib import ExitStack
from dataclasses import dataclass

import numpy as np
import ml_dtypes

import concourse.bacc as bacc
import concourse.mybir as mybir
import concourse.tile as tile

F32 = mybir.dt.float32
BF16 = mybir.dt.bfloat16
FP8 = mybir.dt.float8e4


@dataclass(frozen=True)
class Cfg:
    S: int = 2048          # sequence length (per batch)
    HID: int = 2048        # hidden dim
    QLR: int = 1536        # q lora rank (host-side only)
    KVLR: int = 512        # kv lora rank
    NH_G: int = 4          # heads per core
    DN: int = 128          # nope dim
    DR: int = 64           # rope dim
    DV: int = 128          # v head dim
    ST: int = 512          # phase-1/2 s-tile width
    QT: int = 512          # attention q-tile width

    @property
    def QFN(self):
        return self.NH_G * self.DN      # fused q nope cols (512)

    @property
    def QFR(self):
        return (self.NH_G // 2) * 128   # fused q rope cols, pair-packed (256)

    @property
    def SCALE(self):
        return 1.0 / float(np.sqrt(self.DN + self.DR))


CFG = Cfg()


def build_nc(C: Cfg, reps: int = 1):
    nc = bacc.Bacc("TRN2", target_bir_lowering=False, debug=False, num_devices=8)
    P = 128
    HO = C.HID // P          # 16
    NS = C.S // C.ST         # 4
    KVC = C.KVLR // P        # 4
    NPAIR = C.NH_G // 2      # 2
    NQT = C.S // C.QT        # 4
    NDIAG = C.QT // P        # 4
    NVS = C.S // P           # 16
    NOT = C.HID // 512       # 4
    DV = C.DV

    # ---- kernel I/O ----
    hT = nc.dram_tensor("hT", [C.HID, C.S], BF16, kind="ExternalInput").ap()
    w_qf = nc.dram_tensor("w_qf", [C.HID, C.QFN + C.QFR], BF16,
                          kind="ExternalInput").ap()
    w_kva = nc.dram_tensor("w_kva", [C.HID, C.KVLR], BF16,
                           kind="ExternalInput").ap()
    w_kbn = nc.dram_tensor("w_kbn", [C.KVLR, C.QFN], BF16,
                           kind="ExternalInput").ap()
    w_kbr = nc.dram_tensor("w_kbr", [C.KVLR, C.QFR], BF16,
                           kind="ExternalInput").ap()
    w_vb = nc.dram_tensor("w_vb", [C.KVLR, C.NH_G * DV], BF16,
                          kind="ExternalInput").ap()
    w_ob = nc.dram_tensor("w_ob", [C.NH_G * DV, C.HID], BF16,
                          kind="ExternalInput").ap()
    cos2 = nc.dram_tensor("cos2", [P, C.S], BF16, kind="ExternalInput").ap()
    ssin2 = nc.dram_tensor("ssin2", [P, C.S], BF16, kind="ExternalInput").ap()
    dmask = nc.dram_tensor("dmask", [C.QT, C.QT], BF16, kind="ExternalInput").ap()
    outp = nc.dram_tensor("outp", [C.S, C.HID], F32, kind="ExternalOutput").ap()

    hT_r = hT.rearrange("(ho hi) s -> hi ho s", hi=P)

    with tile.TileContext(nc) as tc:
        for rep in range(reps):
            with ExitStack() as tctx:
                per = tctx.enter_context(tc.tile_pool(name=f"per{rep}", bufs=1))
                # persistent tiles
                cos_sb = per.tile([P, C.S], BF16)
                ssin_sb = per.tile([P, C.S], BF16)
                dm_sb = per.tile([P, NDIAG, C.QT], BF16)
                ones_sb = per.tile([P, P], BF16)
                wqf_sb = per.tile([P, HO, C.QFN + C.QFR], BF16)
                wkva_sb = per.tile([P, HO, C.KVLR], BF16)
                wkn_sb = per.tile([P, KVC, C.QFN], BF16)
                wkr_sb = per.tile([P, KVC, C.QFR], BF16)
                wv_sb = per.tile([P, KVC, C.NH_G * DV], BF16)
                wo_sb = per.tile([P, C.NH_G, C.HID], BF16)
                # q/k in fp8, 2 planes: 0 = nope, 1 = rope (pair-packed)
                qT_sb = per.tile([P, C.NH_G, 2, C.S], FP8)
                kT_sb = per.tile([P, C.NH_G, 2, C.S], FP8)
                v_sb = per.tile([P, NVS, C.NH_G * DV], BF16)

                nc.sync.dma_start(out=cos_sb[:], in_=cos2)
                nc.sync.dma_start(out=ssin_sb[:], in_=ssin2)
                nc.sync.dma_start(
                    out=dm_sb[:], in_=dmask.rearrange("(j ki) q -> ki j q", ki=P))
                nc.vector.memset(ones_sb[:], 1.0)
                wqf_r = w_qf.rearrange("(ho hi) c -> hi ho c", hi=P)
                wkva_r = w_kva.rearrange("(ho hi) c -> hi ho c", hi=P)
                for ho in range(HO):
                    nc.sync.dma_start(out=wqf_sb[:, ho, :], in_=wqf_r[:, ho, :])
                    nc.sync.dma_start(out=wkva_sb[:, ho, :], in_=wkva_r[:, ho, :])
                nc.sync.dma_start(
                    out=wkn_sb[:], in_=w_kbn.rearrange("(co ci) m -> ci co m", ci=P))
                nc.sync.dma_start(
                    out=wkr_sb[:], in_=w_kbr.rearrange("(co ci) m -> ci co m", ci=P))
                nc.sync.dma_start(
                    out=wv_sb[:], in_=w_vb.rearrange("(co ci) m -> ci co m", ci=P))
                nc.sync.dma_start(
                    out=wo_sb[:], in_=w_ob.rearrange("(h d) o -> d h o", d=P))
                # zero the unused rope half of each head's q plane 1 (the k
                # plane 1 keeps the full head pair; the q-side zeros select
                # this head's rope rows in the DoubleRow contraction)
                for h in range(C.NH_G):
                    if h % 2 == 0:
                        nc.vector.memset(qT_sb[64:128, h, 1, :], 0.0)
                    else:
                        nc.vector.memset(qT_sb[0:64, h, 1, :], 0.0)

                ht_pool = tctx.enter_context(tc.tile_pool(name=f"ht{rep}", bufs=2))
                kv_pool = tctx.enter_context(tc.tile_pool(name=f"kv{rep}", bufs=2))
                rp_pool = tctx.enter_context(tc.tile_pool(name=f"rp{rep}", bufs=2))
                e_pool = tctx.enter_context(tc.tile_pool(name=f"e{rep}", bufs=6))
                es_pool = tctx.enter_context(tc.tile_pool(name=f"es{rep}", bufs=2))
                d_pool = tctx.enter_context(tc.tile_pool(name=f"d{rep}", bufs=2))
                ao_pool = tctx.enter_context(tc.tile_pool(name=f"ao{rep}", bufs=2))
                oev_pool = tctx.enter_context(tc.tile_pool(name=f"oe{rep}", bufs=2))
                psA = tctx.enter_context(
                    tc.tile_pool(name=f"psA{rep}", bufs=2, space="PSUM"))
                ps_s = tctx.enter_context(
                    tc.tile_pool(name=f"pss{rep}", bufs=2, space="PSUM"))
                ps_d = tctx.enter_context(
                    tc.tile_pool(name=f"psd{rep}", bufs=1, space="PSUM"))
                ps_o = tctx.enter_context(
                    tc.tile_pool(name=f"pso{rep}", bufs=2, space="PSUM"))
                ps_w = tctx.enter_context(
                    tc.tile_pool(name=f"psw{rep}", bufs=1, space="PSUM"))

                def rope_block(ps_nat, s0, dsts):
                    """RoPE a pair-packed psum block [128, ST] (2 heads x 64
                    rope dims); write f32 results to each (dst_ap, r0, r1)."""
                    tmp = rp_pool.tile([P, C.ST], F32, tag="rtmp")
                    nc.vector.tensor_copy(tmp[:], ps_nat[:])
                    qs = rp_pool.tile([P, C.ST], F32, tag="rqs")
                    for g in range(4):
                        nc.sync.dma_start(
                            out=qs[(g ^ 1) * 32:(g ^ 1) * 32 + 32, :],
                            in_=tmp[g * 32:(g + 1) * 32, :])
                    m1 = rp_pool.tile([P, C.ST], F32, tag="rm1")
                    nc.vector.tensor_mul(m1[:], tmp[:], cos_sb[:, s0:s0 + C.ST])
                    nc.vector.tensor_mul(qs[:], qs[:], ssin_sb[:, s0:s0 + C.ST])
                    for dst_ap, r0, r1 in dsts:
                        nc.vector.tensor_add(dst_ap, m1[r0:r1, :], qs[r0:r1, :])

                for st in range(NS):
                    s0 = st * C.ST
                    sl = slice(s0, s0 + C.ST)

                    # ===== Phase 1: q (fused LoRA) + kv latent from hidden =====
                    ht_sb = ht_pool.tile([P, HO, C.ST], BF16)
                    nc.sync.dma_start(out=ht_sb[:], in_=hT_r[:, :, sl])
                    kv_t = kv_pool.tile([P, KVC, C.ST], BF16)

                    def accum(lhs_sb, col0):
                        ps = psA.tile([P, C.ST], F32, tag="psA")
                        for h in range(HO):
                            nc.tensor.matmul(
                                ps[:], lhs_sb[:, h, col0:col0 + P],
                                ht_sb[:, h, :],
                                start=(h == 0), stop=(h == HO - 1))
                        return ps

                    for t in range(C.NH_G):
                        ps = accum(wqf_sb, t * P)
                        nc.vector.tensor_copy(qT_sb[:, t, 0, sl], ps[:])
                    for pr in range(NPAIR):
                        ps = accum(wqf_sb, C.QFN + pr * P)
                        h0, h1 = 2 * pr, 2 * pr + 1
                        rope_block(ps, s0, [
                            (qT_sb[0:64, h0, 1, sl], 0, 64),
                            (qT_sb[64:128, h1, 1, sl], 64, 128),
                        ])
                    for cc in range(KVC):
                        ps = accum(wkva_sb, cc * P)
                        nc.vector.tensor_copy(kv_t[:, cc, :], ps[:])

                    # ===== Phase 2: k / v head projections from kv latent =====
                    for h in range(C.NH_G):
                        ps = psA.tile([P, C.ST], F32, tag="psA")
                        for cc in range(KVC):
                            nc.tensor.matmul(
                                ps[:], wkn_sb[:, cc, h * C.DN:(h + 1) * C.DN],
                                kv_t[:, cc, :],
                                start=(cc == 0), stop=(cc == KVC - 1))
                        nc.vector.tensor_copy(kT_sb[:, h, 0, sl], ps[:])
                    for pr in range(NPAIR):
                        ps = psA.tile([P, C.ST], F32, tag="psA")
                        for cc in range(KVC):
                            nc.tensor.matmul(
                                ps[:], wkr_sb[:, cc, pr * P:(pr + 1) * P],
                                kv_t[:, cc, :],
                                start=(cc == 0), stop=(cc == KVC - 1))
                        h0, h1 = 2 * pr, 2 * pr + 1
                        rope_block(ps, s0, [(kT_sb[:, h0, 1, sl], 0, 128)])
                        nc.vector.tensor_copy(kT_sb[:, h1, 1, sl],
                                              kT_sb[:, h0, 1, sl])
                    for ssub in range(C.ST // P):
                        vs = (s0 + ssub * P) // P
                        ps = psA.tile([P, C.NH_G * DV], F32, tag="psA")
                        for cc in range(KVC):
                            nc.tensor.matmul(
                                ps[:], kv_t[:, cc, ssub * P:(ssub + 1) * P],
                                wv_sb[:, cc, :],
                                start=(cc == 0), stop=(cc == KVC - 1))
                        nc.vector.tensor_copy(v_sb[:, vs, :], ps[:])

                    # ================= Phase 3: attention (qt = st) ============
                    qt = st
                    q0 = qt * C.QT
                    nkt = (qt + 1) * C.QT // P
                    ao_sb = ao_pool.tile([P, C.NH_G, C.QT], BF16, tag="ao")
                    for h in range(C.NH_G):
                        esum = es_pool.tile([P, C.QT], BF16, tag="es")
                        pso = ps_o.tile([P, C.QT], F32, tag="pso")
                        for kt in range(nkt):
                            j = kt - qt * NDIAG      # >= 0 -> diagonal tile
                            c0 = max(j, 0) * P       # first unmasked column
                            k0 = kt * P
                            pss = ps_s.tile([P, C.QT], F32, tag="pss")
                            nc.tensor.matmul(
                                pss[:, c0:], kT_sb[:, h, :, k0:k0 + P],
                                qT_sb[:, h, :, q0 + c0:q0 + C.QT],
                                start=True, stop=True,
                                perf_mode=mybir.MatmulPerfMode.DoubleRow)
                            e = e_pool.tile([P, C.QT], BF16, tag="e")
                            nc.scalar.activation(
                                e[:, c0:], pss[:, c0:],
                                mybir.ActivationFunctionType.Exp, scale=C.SCALE)
                            if j >= 0:
                                nc.vector.tensor_mul(
                                    e[:, c0:c0 + P], e[:, c0:c0 + P],
                                    dm_sb[:, j, c0:c0 + P])
                            if kt == 0:
                                nc.vector.tensor_copy(esum[:], e[:])
                            else:
                                nc.vector.tensor_add(
                                    esum[:, c0:], esum[:, c0:], e[:, c0:])
                            nc.tensor.matmul(
                                pso[:, c0:], v_sb[:, kt, h * DV:(h + 1) * DV],
                                e[:, c0:],
                                start=(kt == 0), stop=(kt == nkt - 1))
                        psd = ps_d.tile([P, C.QT], F32, tag="psd")
                        nc.tensor.matmul(psd[:], ones_sb[:], esum[:],
                                         start=True, stop=True)
                        rec = d_pool.tile([P, C.QT], F32, tag="rec")
                        nc.vector.reciprocal(rec[:], psd[:])
                        nc.vector.tensor_mul(ao_sb[:, h, :], pso[:], rec[:])
                    for qs in range(C.QT // P):
                        for ot in range(NOT):
                            psw = ps_w.tile([P, 512], F32, tag="psw")
                            for h in range(C.NH_G):
                                nc.tensor.matmul(
                                    psw[:], ao_sb[:, h, qs * P:(qs + 1) * P],
                                    wo_sb[:, h, ot * 512:(ot + 1) * 512],
                                    start=(h == 0), stop=(h == C.NH_G - 1))
                            oev = oev_pool.tile([P, 512], F32)
                            nc.vector.tensor_copy(oev[:], psw[:])
                            nc.sync.dma_start(
                                out=outp[q0 + qs * P:q0 + (qs + 1) * P,
                                         ot * 512:(ot + 1) * 512],
                                in_=oev[:])

    nc.compile()
    return nc


def rope_tables(C: Cfg):
    """cos2/ssin2 [128, S] bf16: two stacked 64-row blocks (head pairs
    share); ssin has the rotate-half sign baked into the first 32 rows."""
    inv = 1.0 / (10000.0 ** (np.arange(0, C.DR, 2, dtype=np.float64) / C.DR))
    freqs = np.arange(C.S, dtype=np.float64)[:, None] * inv[None, :]  # [S, 32]
    emb = np.concatenate([freqs, freqs], axis=1)  # [S, 64]
    cos = np.cos(emb).T.astype(np.float32)   # [64, S]
    sin = np.sin(emb).T.astype(np.float32)
    ssin = sin.copy()
    ssin[: C.DR // 2] = -ssin[: C.DR // 2]
    cos2 = np.concatenate([cos, cos], axis=0)     # [128, S]
    ssin2 = np.concatenate([ssin, ssin], axis=0)
    bf = lambda x: np.ascontiguousarray(x).astype(ml_dtypes.bfloat16)
    return bf(cos2), bf(ssin2)


_FUSED_CACHE = {}


def _fused_wq(inputs):
    """w_q_a @ [w_q_nope | w_q_rope] for all heads, computed once."""
    key = id(inputs.get("w_q_a"))
    if _FUSED_CACHE.get("key") != key:
        w_q_a = np.asarray(inputs["w_q_a"], dtype=np.float32)
        wn = w_q_a @ np.asarray(inputs["w_q_nope"], dtype=np.float32)
        wr = w_q_a @ np.asarray(inputs["w_q_rope"], dtype=np.float32)
        _FUSED_CACHE.update(key=key, wn=wn, wr=wr)
    return _FUSED_CACHE["wn"], _FUSED_CACHE["wr"]


def host_inputs(C: Cfg, inputs: dict, core: int):
    """Build the per-core input map from full inputs."""
    NH = inputs["w_q_nope"].shape[1] // C.DN
    groups = NH // C.NH_G
    b = core // groups
    g = core % groups
    hs = slice(g * C.NH_G, (g + 1) * C.NH_G)

    bf = lambda x: np.ascontiguousarray(
        np.asarray(x, dtype=np.float32)).astype(ml_dtypes.bfloat16)

    wn_full, wr_full = _fused_wq(inputs)
    w_qfn = wn_full.reshape(C.HID, NH, C.DN)[:, hs].reshape(C.HID, -1)
    w_qfr = wr_full.reshape(C.HID, NH, C.DR)[:, hs].reshape(C.HID, -1)
    w_qf = bf(np.concatenate([w_qfn, w_qfr], axis=1))

    hT = bf(inputs["hidden_states"][b].T)
    w_kva = bf(inputs["w_kv_a"])
    w_kbn = bf(inputs["w_k_nope"].reshape(C.KVLR, NH, C.DN)[:, hs]
               .reshape(C.KVLR, -1))
    w_kbr = bf(inputs["w_k_rope"].reshape(C.KVLR, NH, C.DR)[:, hs]
               .reshape(C.KVLR, -1))
    w_vb = bf(inputs["w_v"].reshape(C.KVLR, NH, C.DV)[:, hs]
              .reshape(C.KVLR, -1))
    w_ob = bf(inputs["w_o"].reshape(NH, C.DV, C.HID)[hs].reshape(-1, C.HID))
    cos2, ssin2 = rope_tables(C)
    cm = np.asarray(inputs["causal_mask"])[0, 0]
    dmask = np.ascontiguousarray(
        cm[-C.QT:, -C.QT:].T.astype(np.float32)).astype(ml_dtypes.bfloat16)
    return {
        "hT": hT, "w_qf": w_qf, "w_kva": w_kva,
        "w_kbn": w_kbn, "w_kbr": w_kbr, "w_vb": w_vb, "w_ob": w_ob,
        "cos2": cos2, "ssin2": ssin2, "dmask": dmask,
    }


_NC_CACHE = {}


def kernel(**inputs) -> np.ndarray:
    from concourse.bass_utils import run_bass_kernel_spmd

    C = CFG
    if "nc" not in _NC_CACHE:
        _NC_CACHE["nc"] = build_nc(C)
    nc = _NC_CACHE["nc"]

    in_maps = [host_inputs(C, inputs, c) for c in range(8)]
    res = run_bass_kernel_spmd(nc, in_maps, core_ids=list(range(8)))

    B = inputs["hidden_states"].shape[0]
    groups = 8 // B
    out = np.zeros((B, C.S, C.HID), dtype=np.float32)
    for c in range(8):
        out[c // groups] += res.results[c]["outp"]
    return out


# revision 13
# speedup vs baseline: 1.7000x; 1.0034x over previous
"""DeepseekV3 MLA attention kernel for 8 Trainium2 NeuronCores — v2.

Sharding: 2-way data-parallel over batch x 4-way tensor-parallel over heads.
Core c handles batch b = c // 4 and heads [4*(c%4) .. 4*(c%4)+4).

v2 design vs baseline:
  - all projection / AV / w_o matmuls in bf16 (same PE rate as fp32r but
    half the SBUF/DMA traffic, FWL weight loads, 2x DVE elementwise)
  - score matmuls in fp8e4 with DoubleRow perf mode: nope(128) + rope(64,
    zero-padded) packed as a 2-plane 256-deep contraction -> one matmul
    at 0.5 cycles/row
  - softmax denominators via DVE running-sum of e-tiles + one ones-matmul
    per (head, q-tile) instead of a ones-matmul per (head, k-tile)
  - causal diag shrink: score/exp/AV restricted to the unmasked column
    range on diagonal tiles
  - all intermediates (q, k, v) SBUF-resident, no DRAM scratch
  - phases software-pipelined per 512-wide s-tile:
    p1(st) -> p2(st) -> attention(qt=st)

All tolerances validated numerically on the CPU reference: bf16-everything
gives 4.3e-3 scale-relative max error; fp8 q/k adds ~6.7e-3 (tolerance 2e-2).
"""

from contextlib import ExitStack
from dataclasses import dataclass

import numpy as np
import ml_dtypes

import concourse.bacc as bacc
import concourse.mybir as mybir
import concourse.tile as tile

F32 = mybir.dt.float32
BF16 = mybir.dt.bfloat16
FP8 = mybir.dt.float8e4


@dataclass(frozen=True)
class Cfg:
    S: int = 2048          # sequence length (per batch)
    HID: int = 2048        # hidden dim
    QLR: int = 1536        # q lora rank (host-side only)
    KVLR: int = 512        # kv lora rank
    NH_G: int = 4          # heads per core
    DN: int = 128          # nope dim
    DR: int = 64           # rope dim
    DV: int = 128          # v head dim
    ST: int = 512          # phase-1/2 s-tile width
    QT: int = 512          # attention q-tile width

    @property
    def QFN(self):
        return self.NH_G * self.DN      # fused q nope cols (512)

    @property
    def QFR(self):
        return (self.NH_G // 2) * 128   # fused q rope cols, pair-packed (256)

    @property
    def SCALE(self):
        return 1.0 / float(np.sqrt(self.DN + self.DR))


CFG = Cfg()


def build_nc(C: Cfg, reps: int = 1):
    nc = bacc.Bacc("TRN2", target_bir_lowering=False, debug=False, num_devices=8)
    P = 128
    HO = C.HID // P          # 16
    NS = C.S // C.ST         # 4
    KVC = C.KVLR // P        # 4
    NPAIR = C.NH_G // 2      # 2
    NQT = C.S // C.QT        # 4
    NDIAG = C.QT // P        # 4
    NVS = C.S // P           # 16
    NOT = C.HID // 512       # 4
    DV = C.DV

    # ---- kernel I/O ----
    hT = nc.dram_tensor("hT", [C.HID, C.S], BF16, kind="ExternalInput").ap()
    w_qf = nc.dram_tensor("w_qf", [C.HID, C.QFN + C.QFR], BF16,
                          kind="ExternalInput").ap()
    w_kva = nc.dram_tensor("w_kva", [C.HID, C.KVLR], BF16,
                           kind="ExternalInput").ap()
    w_kbn = nc.dram_tensor("w_kbn", [C.KVLR, C.QFN], BF16,
                           kind="ExternalInput").ap()
    w_kbr = nc.dram_tensor("w_kbr", [C.KVLR, C.QFR], BF16,
                           kind="ExternalInput").ap()
    w_vb = nc.dram_tensor("w_vb", [C.KVLR, C.NH_G * DV], BF16,
                          kind="ExternalInput").ap()
    w_ob = nc.dram_tensor("w_ob", [C.NH_G * DV, C.HID], BF16,
                          kind="ExternalInput").ap()
    cos2 = nc.dram_tensor("cos2", [P, C.S], BF16, kind="ExternalInput").ap()
    ssin2 = nc.dram_tensor("ssin2", [P, C.S], BF16, kind="ExternalInput").ap()
    dmask = nc.dram_tensor("dmask", [C.QT, C.QT], BF16, kind="ExternalInput").ap()
    outp = nc.dram_tensor("outp", [C.S, C.HID], BF16, kind="ExternalOutput").ap()

    hT_r = hT.rearrange("(ho hi) s -> hi ho s", hi=P)

    with tile.TileContext(nc) as tc:
        for rep in range(reps):
            with ExitStack() as tctx:
                per = tctx.enter_context(tc.tile_pool(name=f"per{rep}", bufs=1))
                ht_pool = tctx.enter_context(tc.tile_pool(name=f"ht{rep}", bufs=2))
                # persistent tiles
                cos_sb = per.tile([P, C.S], BF16)
                ssin_sb = per.tile([P, C.S], BF16)
                dm_sb = per.tile([P, NDIAG, C.QT], BF16)
                ones_sb = per.tile([P, P], BF16)
                wqf_sb = [per.tile([P, C.QFN + C.QFR], BF16, tag=f"wqf{ho}",
                                   name=f"wqf{ho}") for ho in range(HO)]
                wkva_sb = [per.tile([P, C.KVLR], BF16, tag=f"wkva{ho}",
                                    name=f"wkva{ho}") for ho in range(HO)]
                wkn_sb = per.tile([P, KVC, C.QFN], BF16)
                wkr_sb = per.tile([P, KVC, C.QFR], BF16)
                wv_sb = per.tile([P, KVC, C.NH_G * DV], BF16)
                wo_sb = per.tile([P, C.NH_G, C.HID], BF16)
                # q/k in fp8, 2 planes: 0 = nope, 1 = rope (pair-packed)
                qT_sb = per.tile([P, C.NH_G, 2, C.S], FP8)
                kT_sb = per.tile([P, C.NH_G, 2, C.S], FP8)
                v_sb = per.tile([P, NVS, C.NH_G * DV], BF16)

                # hT prefetch: st=0 queued before any weight DMA so the
                # first accumulation can start immediately
                ht_tiles = {}

                def load_ht(st):
                    t = ht_pool.tile([P, HO, C.ST], BF16, tag="ht")
                    s = slice(st * C.ST, (st + 1) * C.ST)
                    for ho in range(0, HO, 2):
                        nc.sync.dma_start(
                            out=t[:, ho:ho + 2, :], in_=hT_r[:, ho:ho + 2, s])
                    ht_tiles[st] = t

                load_ht(0)
                wqf_r = w_qf.rearrange("(ho hi) c -> hi ho c", hi=P)
                wkva_r = w_kva.rearrange("(ho hi) c -> hi ho c", hi=P)
                for ho in range(HO):
                    nc.sync.dma_start(out=wqf_sb[ho][:], in_=wqf_r[:, ho, :])
                for ho in range(HO):
                    nc.sync.dma_start(out=wkva_sb[ho][:], in_=wkva_r[:, ho, :])
                nc.sync.dma_start(out=cos_sb[:], in_=cos2)
                nc.sync.dma_start(out=ssin_sb[:], in_=ssin2)
                nc.gpsimd.memset(ones_sb[:], 1.0)
                # zero the unused rope half of each head's q plane 1 (the k
                # plane 1 keeps the full head pair; the q-side zeros select
                # this head's rope rows in the DoubleRow contraction)
                for h in range(C.NH_G):
                    if h % 2 == 0:
                        nc.gpsimd.memset(qT_sb[64:128, h, 1, :], 0.0)
                    else:
                        nc.gpsimd.memset(qT_sb[0:64, h, 1, :], 0.0)
                kv_pool = tctx.enter_context(tc.tile_pool(name=f"kv{rep}", bufs=2))
                rp_pool = tctx.enter_context(tc.tile_pool(name=f"rp{rep}", bufs=2))
                e_pool = tctx.enter_context(tc.tile_pool(name=f"e{rep}", bufs=6))
                es_pool = tctx.enter_context(tc.tile_pool(name=f"es{rep}", bufs=2))
                d_pool = tctx.enter_context(tc.tile_pool(name=f"d{rep}", bufs=2))
                ao_pool = tctx.enter_context(tc.tile_pool(name=f"ao{rep}", bufs=2))
                oev_pool = tctx.enter_context(tc.tile_pool(name=f"oe{rep}", bufs=2))
                psA = tctx.enter_context(
                    tc.tile_pool(name=f"psA{rep}", bufs=2, space="PSUM"))
                ps_s = tctx.enter_context(
                    tc.tile_pool(name=f"pss{rep}", bufs=2, space="PSUM"))
                ps_d = tctx.enter_context(
                    tc.tile_pool(name=f"psd{rep}", bufs=1, space="PSUM"))
                ps_o = tctx.enter_context(
                    tc.tile_pool(name=f"pso{rep}", bufs=2, space="PSUM"))
                ps_w = tctx.enter_context(
                    tc.tile_pool(name=f"psw{rep}", bufs=1, space="PSUM"))

                def rope_block(ps_nat, s0, dsts):
                    """RoPE a pair-packed psum block [128, ST] (2 heads x 64
                    rope dims); write f32 results to each (dst_ap, r0, r1)."""
                    tmp = rp_pool.tile([P, C.ST], F32, tag="rtmp")
                    nc.vector.tensor_copy(tmp[:], ps_nat[:])
                    qs = rp_pool.tile([P, C.ST], F32, tag="rqs")
                    for g in range(4):
                        nc.sync.dma_start(
                            out=qs[(g ^ 1) * 32:(g ^ 1) * 32 + 32, :],
                            in_=tmp[g * 32:(g + 1) * 32, :])
                    m1 = rp_pool.tile([P, C.ST], F32, tag="rm1")
                    nc.vector.tensor_mul(m1[:], tmp[:], cos_sb[:, s0:s0 + C.ST])
                    nc.vector.tensor_mul(qs[:], qs[:], ssin_sb[:, s0:s0 + C.ST])
                    for dst_ap, r0, r1 in dsts:
                        nc.vector.tensor_add(dst_ap, m1[r0:r1, :], qs[r0:r1, :])

                for st in range(NS):
                    s0 = st * C.ST
                    sl = slice(s0, s0 + C.ST)

                    # ===== Phase 1: q (fused LoRA) + kv latent from hidden =====
                    ht_sb = ht_tiles.pop(st)
                    if st + 1 < NS:
                        load_ht(st + 1)
                    kv_t = kv_pool.tile([P, KVC, C.ST], BF16)

                    def accum(lhs_sb, col0):
                        ps = psA.tile([P, C.ST], F32, tag="psA")
                        for h in range(HO):
                            nc.tensor.matmul(
                                ps[:], lhs_sb[h][:, col0:col0 + P],
                                ht_sb[:, h, :],
                                start=(h == 0), stop=(h == HO - 1))
                        return ps

                    for t in range(C.NH_G):
                        ps = accum(wqf_sb, t * P)
                        nc.any.tensor_copy(qT_sb[:, t, 0, sl], ps[:])
                    for pr in range(NPAIR):
                        ps = accum(wqf_sb, C.QFN + pr * P)
                        h0, h1 = 2 * pr, 2 * pr + 1
                        rope_block(ps, s0, [
                            (qT_sb[0:64, h0, 1, sl], 0, 64),
                            (qT_sb[64:128, h1, 1, sl], 64, 128),
                        ])
                    for cc in range(KVC):
                        ps = accum(wkva_sb, cc * P)
                        nc.any.tensor_copy(kv_t[:, cc, :], ps[:])

                    if st == 0:
                        nc.sync.dma_start(
                            out=wkn_sb[:],
                            in_=w_kbn.rearrange("(co ci) m -> ci co m", ci=P))
                        nc.sync.dma_start(
                            out=wkr_sb[:],
                            in_=w_kbr.rearrange("(co ci) m -> ci co m", ci=P))
                        nc.sync.dma_start(
                            out=wv_sb[:],
                            in_=w_vb.rearrange("(co ci) m -> ci co m", ci=P))

                    # ===== Phase 2: k / v head projections from kv latent =====
                    for h in range(C.NH_G):
                        ps = psA.tile([P, C.ST], F32, tag="psA")
                        for cc in range(KVC):
                            nc.tensor.matmul(
                                ps[:], wkn_sb[:, cc, h * C.DN:(h + 1) * C.DN],
                                kv_t[:, cc, :],
                                start=(cc == 0), stop=(cc == KVC - 1))
                        nc.any.tensor_copy(kT_sb[:, h, 0, sl], ps[:])
                    for pr in range(NPAIR):
                        ps = psA.tile([P, C.ST], F32, tag="psA")
                        for cc in range(KVC):
                            nc.tensor.matmul(
                                ps[:], wkr_sb[:, cc, pr * P:(pr + 1) * P],
                                kv_t[:, cc, :],
                                start=(cc == 0), stop=(cc == KVC - 1))
                        h0, h1 = 2 * pr, 2 * pr + 1
                        rope_block(ps, s0, [(kT_sb[:, h0, 1, sl], 0, 128)])
                        nc.any.tensor_copy(kT_sb[:, h1, 1, sl],
                                           kT_sb[:, h0, 1, sl])
                    for ssub in range(C.ST // P):
                        vs = (s0 + ssub * P) // P
                        ps = psA.tile([P, C.NH_G * DV], F32, tag="psA")
                        for cc in range(KVC):
                            nc.tensor.matmul(
                                ps[:], kv_t[:, cc, ssub * P:(ssub + 1) * P],
                                wv_sb[:, cc, :],
                                start=(cc == 0), stop=(cc == KVC - 1))
                        nc.any.tensor_copy(v_sb[:, vs, :], ps[:])

                    if st == 0:
                        nc.sync.dma_start(
                            out=wo_sb[:],
                            in_=w_ob.rearrange("(h d) o -> d h o", d=P))
                        nc.sync.dma_start(
                            out=dm_sb[:],
                            in_=dmask.rearrange("(j ki) q -> ki j q", ki=P))

                    # ================= Phase 3: attention (qt = st) ============
                    qt = st
                    q0 = qt * C.QT
                    nkt = (qt + 1) * C.QT // P
                    ao_sb = ao_pool.tile([P, C.NH_G, C.QT], BF16, tag="ao")
                    for h in range(C.NH_G):
                        # two interleaved partial e-sums keep the DVE
                        # accumulation chains short
                        esum = [es_pool.tile([P, C.QT], BF16, tag="esA",
                                             name="esumA"),
                                es_pool.tile([P, C.QT], BF16, tag="esB",
                                             name="esumB")]
                        pso = ps_o.tile([P, C.QT], F32, tag="pso")
                        for kt in range(nkt):
                            j = kt - qt * NDIAG      # >= 0 -> diagonal tile
                            c0 = max(j, 0) * P       # first unmasked column
                            k0 = kt * P
                            pss = ps_s.tile([P, C.QT], F32, tag="pss")
                            nc.tensor.matmul(
                                pss[:, c0:], kT_sb[:, h, :, k0:k0 + P],
                                qT_sb[:, h, :, q0 + c0:q0 + C.QT],
                                start=True, stop=True,
                                perf_mode=mybir.MatmulPerfMode.DoubleRow)
                            e = e_pool.tile([P, C.QT], BF16, tag="e")
                            nc.scalar.activation(
                                e[:, c0:], pss[:, c0:],
                                mybir.ActivationFunctionType.Exp, scale=C.SCALE)
                            if j >= 0:
                                nc.vector.tensor_mul(
                                    e[:, c0:c0 + P], e[:, c0:c0 + P],
                                    dm_sb[:, j, c0:c0 + P])
                            es = esum[kt % 2]
                            if kt < 2:
                                if c0 > 0:
                                    nc.vector.memset(es[:, :c0], 0.0)
                                nc.vector.tensor_copy(es[:, c0:], e[:, c0:])
                            else:
                                nc.vector.tensor_add(
                                    es[:, c0:], es[:, c0:], e[:, c0:])
                            nc.tensor.matmul(
                                pso[:, c0:], v_sb[:, kt, h * DV:(h + 1) * DV],
                                e[:, c0:],
                                start=(kt == 0), stop=(kt == nkt - 1))
                        psd = ps_d.tile([P, C.QT], F32, tag="psd")
                        nterm = min(nkt, 2)
                        for i in range(nterm):
                            nc.tensor.matmul(psd[:], ones_sb[:], esum[i][:],
                                             start=(i == 0),
                                             stop=(i == nterm - 1))
                        rec = d_pool.tile([P, C.QT], F32, tag="rec")
                        nc.vector.reciprocal(rec[:], psd[:])
                        nc.vector.tensor_mul(ao_sb[:, h, :], pso[:], rec[:])
                    for qs in range(C.QT // P):
                        for ot in range(NOT):
                            psw = ps_w.tile([P, 512], F32, tag="psw")
                            for h in range(C.NH_G):
                                nc.tensor.matmul(
                                    psw[:], ao_sb[:, h, qs * P:(qs + 1) * P],
                                    wo_sb[:, h, ot * 512:(ot + 1) * 512],
                                    start=(h == 0), stop=(h == C.NH_G - 1))
                            oev = oev_pool.tile([P, 512], BF16)
                            nc.any.tensor_copy(oev[:], psw[:])
                            nc.sync.dma_start(
                                out=outp[q0 + qs * P:q0 + (qs + 1) * P,
                                         ot * 512:(ot + 1) * 512],
                                in_=oev[:])

    nc.compile()
    return nc


def rope_tables(C: Cfg):
    """cos2/ssin2 [128, S] bf16: two stacked 64-row blocks (head pairs
    share); ssin has the rotate-half sign baked into the first 32 rows."""
    inv = 1.0 / (10000.0 ** (np.arange(0, C.DR, 2, dtype=np.float64) / C.DR))
    freqs = np.arange(C.S, dtype=np.float64)[:, None] * inv[None, :]  # [S, 32]
    emb = np.concatenate([freqs, freqs], axis=1)  # [S, 64]
    cos = np.cos(emb).T.astype(np.float32)   # [64, S]
    sin = np.sin(emb).T.astype(np.float32)
    ssin = sin.copy()
    ssin[: C.DR // 2] = -ssin[: C.DR // 2]
    cos2 = np.concatenate([cos, cos], axis=0)     # [128, S]
    ssin2 = np.concatenate([ssin, ssin], axis=0)
    bf = lambda x: np.ascontiguousarray(x).astype(ml_dtypes.bfloat16)
    return bf(cos2), bf(ssin2)


_FUSED_CACHE = {}


def _fused_wq(inputs):
    """w_q_a @ [w_q_nope | w_q_rope] for all heads, computed once."""
    key = id(inputs.get("w_q_a"))
    if _FUSED_CACHE.get("key") != key:
        w_q_a = np.asarray(inputs["w_q_a"], dtype=np.float32)
        wn = w_q_a @ np.asarray(inputs["w_q_nope"], dtype=np.float32)
        wr = w_q_a @ np.asarray(inputs["w_q_rope"], dtype=np.float32)
        _FUSED_CACHE.update(key=key, wn=wn, wr=wr)
    return _FUSED_CACHE["wn"], _FUSED_CACHE["wr"]


def host_inputs(C: Cfg, inputs: dict, core: int):
    """Build the per-core input map from full inputs."""
    NH = inputs["w_q_nope"].shape[1] // C.DN
    groups = NH // C.NH_G
    b = core // groups
    g = core % groups
    hs = slice(g * C.NH_G, (g + 1) * C.NH_G)

    bf = lambda x: np.ascontiguousarray(
        np.asarray(x, dtype=np.float32)).astype(ml_dtypes.bfloat16)

    wn_full, wr_full = _fused_wq(inputs)
    w_qfn = wn_full.reshape(C.HID, NH, C.DN)[:, hs].reshape(C.HID, -1)
    w_qfr = wr_full.reshape(C.HID, NH, C.DR)[:, hs].reshape(C.HID, -1)
    w_qf = bf(np.concatenate([w_qfn, w_qfr], axis=1))

    hT = bf(inputs["hidden_states"][b].T)
    w_kva = bf(inputs["w_kv_a"])
    w_kbn = bf(inputs["w_k_nope"].reshape(C.KVLR, NH, C.DN)[:, hs]
               .reshape(C.KVLR, -1))
    w_kbr = bf(inputs["w_k_rope"].reshape(C.KVLR, NH, C.DR)[:, hs]
               .reshape(C.KVLR, -1))
    w_vb = bf(inputs["w_v"].reshape(C.KVLR, NH, C.DV)[:, hs]
              .reshape(C.KVLR, -1))
    w_ob = bf(inputs["w_o"].reshape(NH, C.DV, C.HID)[hs].reshape(-1, C.HID))
    cos2, ssin2 = rope_tables(C)
    cm = np.asarray(inputs["causal_mask"])[0, 0]
    dmask = np.ascontiguousarray(
        cm[-C.QT:, -C.QT:].T.astype(np.float32)).astype(ml_dtypes.bfloat16)
    return {
        "hT": hT, "w_qf": w_qf, "w_kva": w_kva,
        "w_kbn": w_kbn, "w_kbr": w_kbr, "w_vb": w_vb, "w_ob": w_ob,
        "cos2": cos2, "ssin2": ssin2, "dmask": dmask,
    }


_NC_CACHE = {}


def kernel(**inputs) -> np.ndarray:
    from concourse.bass_utils import run_bass_kernel_spmd

    C = CFG
    if "nc" not in _NC_CACHE:
        _NC_CACHE["nc"] = build_nc(C)
    nc = _NC_CACHE["nc"]

    in_maps = [host_inputs(C, inputs, c) for c in range(8)]
    res = run_bass_kernel_spmd(nc, in_maps, core_ids=list(range(8)))

    B = inputs["hidden_states"].shape[0]
    groups = 8 // B
    out = np.zeros((B, C.S, C.HID), dtype=np.float32)
    for c in range(8):
        out[c // groups] += np.asarray(res.results[c]["outp"],
                                       dtype=np.float32)
    return out


# revision 14
# speedup vs baseline: 1.7214x; 1.0126x over previous
"""DeepseekV3 MLA attention kernel for 8 Trainium2 NeuronCores — v2.

Sharding: 2-way data-parallel over batch x 4-way tensor-parallel over heads.
Core c handles batch b = c // 4 and heads [4*(c%4) .. 4*(c%4)+4).

v2 design vs baseline:
  - all projection / AV / w_o matmuls in bf16 (same PE rate as fp32r but
    half the SBUF/DMA traffic, FWL weight loads, 2x DVE elementwise)
  - score matmuls in fp8e4 with DoubleRow perf mode: nope(128) + rope(64,
    zero-padded) packed as a 2-plane 256-deep contraction -> one matmul
    at 0.5 cycles/row
  - softmax denominators via DVE running-sum of e-tiles + one ones-matmul
    per (head, q-tile) instead of a ones-matmul per (head, k-tile)
  - causal diag shrink: score/exp/AV restricted to the unmasked column
    range on diagonal tiles
  - all intermediates (q, k, v) SBUF-resident, no DRAM scratch
  - phases software-pipelined per 512-wide s-tile:
    p1(st) -> p2(st) -> attention(qt=st)

All tolerances validated numerically on the CPU reference: bf16-everything
gives 4.3e-3 scale-relative max error; fp8 q/k adds ~6.7e-3 (tolerance 2e-2).
"""

from contextlib import ExitStack
from dataclasses import dataclass

import numpy as np
import ml_dtypes

import concourse.bacc as bacc
import concourse.mybir as mybir
import concourse.tile as tile

F32 = mybir.dt.float32
BF16 = mybir.dt.bfloat16
FP8 = mybir.dt.float8e4


@dataclass(frozen=True)
class Cfg:
    S: int = 2048          # sequence length (per batch)
    HID: int = 2048        # hidden dim
    QLR: int = 1536        # q lora rank (host-side only)
    KVLR: int = 512        # kv lora rank
    NH_G: int = 4          # heads per core
    DN: int = 128          # nope dim
    DR: int = 64           # rope dim
    DV: int = 128          # v head dim
    ST: int = 512          # phase-1/2 s-tile width
    QT: int = 512          # attention q-tile width

    @property
    def QFN(self):
        return self.NH_G * self.DN      # fused q nope cols (512)

    @property
    def QFR(self):
        return (self.NH_G // 2) * 128   # fused q rope cols, pair-packed (256)

    @property
    def SCALE(self):
        return 1.0 / float(np.sqrt(self.DN + self.DR))


CFG = Cfg()


def build_nc(C: Cfg, reps: int = 1):
    nc = bacc.Bacc("TRN2", target_bir_lowering=False, debug=False, num_devices=8)
    P = 128
    HO = C.HID // P          # 16
    NS = C.S // C.ST         # 4
    KVC = C.KVLR // P        # 4
    NPAIR = C.NH_G // 2      # 2
    NQT = C.S // C.QT        # 4
    NDIAG = C.QT // P        # 4
    NVS = C.S // P           # 16
    NOT = C.HID // 512       # 4
    DV = C.DV

    # ---- kernel I/O ----
    hT = nc.dram_tensor("hT", [C.HID, C.S], BF16, kind="ExternalInput").ap()
    w_qf = nc.dram_tensor("w_qf", [C.HID, C.QFN + C.QFR], BF16,
                          kind="ExternalInput").ap()
    w_kva = nc.dram_tensor("w_kva", [C.HID, C.KVLR], BF16,
                           kind="ExternalInput").ap()
    w_kbn = nc.dram_tensor("w_kbn", [C.KVLR, C.QFN], BF16,
                           kind="ExternalInput").ap()
    w_kbr = nc.dram_tensor("w_kbr", [C.KVLR, C.QFR], BF16,
                           kind="ExternalInput").ap()
    w_vb = nc.dram_tensor("w_vb", [C.KVLR, C.NH_G * DV], BF16,
                          kind="ExternalInput").ap()
    w_ob = nc.dram_tensor("w_ob", [C.NH_G * DV, C.HID], BF16,
                          kind="ExternalInput").ap()
    cos2 = nc.dram_tensor("cos2", [P, C.S], BF16, kind="ExternalInput").ap()
    ssin2 = nc.dram_tensor("ssin2", [P, C.S], BF16, kind="ExternalInput").ap()
    dmask = nc.dram_tensor("dmask", [C.QT, C.QT], BF16, kind="ExternalInput").ap()
    outp = nc.dram_tensor("outp", [C.S, C.HID], BF16, kind="ExternalOutput").ap()

    hT_r = hT.rearrange("(ho hi) s -> hi ho s", hi=P)

    with tile.TileContext(nc) as tc:
        for rep in range(reps):
            with ExitStack() as tctx:
                per = tctx.enter_context(tc.tile_pool(name=f"per{rep}", bufs=1))
                ht_pool = tctx.enter_context(tc.tile_pool(name=f"ht{rep}", bufs=2))
                # persistent tiles
                cos_sb = per.tile([P, C.S], BF16)
                ssin_sb = per.tile([P, C.S], BF16)
                dm_sb = per.tile([P, NDIAG, C.QT], BF16)
                ones_sb = per.tile([P, P], BF16)
                wqf_sb = [per.tile([P, C.QFN + C.QFR], BF16, tag=f"wqf{ho}",
                                   name=f"wqf{ho}") for ho in range(HO)]
                wkva_sb = [per.tile([P, C.KVLR], BF16, tag=f"wkva{ho}",
                                    name=f"wkva{ho}") for ho in range(HO)]
                wkn_sb = per.tile([P, KVC, C.QFN], BF16)
                wkr_sb = per.tile([P, KVC, C.QFR], BF16)
                wv_sb = per.tile([P, KVC, C.NH_G * DV], BF16)
                wo_sb = per.tile([P, C.NH_G, C.HID], BF16)
                # q/k in fp8, 2 planes: 0 = nope, 1 = rope (pair-packed)
                qT_sb = per.tile([P, C.NH_G, 2, C.S], FP8)
                kT_sb = per.tile([P, C.NH_G, 2, C.S], FP8)
                v_sb = per.tile([P, NVS, C.NH_G * DV], BF16)

                # hT prefetch: st=0 queued before any weight DMA so the
                # first accumulation can start immediately
                ht_tiles = {}

                def load_ht(st):
                    t = ht_pool.tile([P, HO, C.ST], BF16, tag="ht")
                    s = slice(st * C.ST, (st + 1) * C.ST)
                    for ho in range(0, HO, 2):
                        nc.sync.dma_start(
                            out=t[:, ho:ho + 2, :], in_=hT_r[:, ho:ho + 2, s])
                    ht_tiles[st] = t

                load_ht(0)
                wqf_r = w_qf.rearrange("(ho hi) c -> hi ho c", hi=P)
                wkva_r = w_kva.rearrange("(ho hi) c -> hi ho c", hi=P)
                for ho in range(HO):
                    nc.sync.dma_start(out=wqf_sb[ho][:], in_=wqf_r[:, ho, :])
                for ho in range(HO):
                    nc.sync.dma_start(out=wkva_sb[ho][:], in_=wkva_r[:, ho, :])
                nc.sync.dma_start(out=cos_sb[:], in_=cos2)
                nc.sync.dma_start(out=ssin_sb[:], in_=ssin2)
                nc.gpsimd.memset(ones_sb[:], 1.0)
                # zero the unused rope half of each head's q plane 1 (the k
                # plane 1 keeps the full head pair; the q-side zeros select
                # this head's rope rows in the DoubleRow contraction)
                for h in range(C.NH_G):
                    if h % 2 == 0:
                        nc.gpsimd.memset(qT_sb[64:128, h, 1, :], 0.0)
                    else:
                        nc.gpsimd.memset(qT_sb[0:64, h, 1, :], 0.0)
                kv_pool = tctx.enter_context(tc.tile_pool(name=f"kv{rep}", bufs=2))
                rp_pool = tctx.enter_context(tc.tile_pool(name=f"rp{rep}", bufs=2))
                e_pool = tctx.enter_context(tc.tile_pool(name=f"e{rep}", bufs=6))
                es_pool = tctx.enter_context(tc.tile_pool(name=f"es{rep}", bufs=2))
                d_pool = tctx.enter_context(tc.tile_pool(name=f"d{rep}", bufs=2))
                ao_pool = tctx.enter_context(tc.tile_pool(name=f"ao{rep}", bufs=2))
                oev_pool = tctx.enter_context(tc.tile_pool(name=f"oe{rep}", bufs=2))
                psA = tctx.enter_context(
                    tc.tile_pool(name=f"psA{rep}", bufs=2, space="PSUM"))
                ps_s = tctx.enter_context(
                    tc.tile_pool(name=f"pss{rep}", bufs=2, space="PSUM"))
                ps_d = tctx.enter_context(
                    tc.tile_pool(name=f"psd{rep}", bufs=1, space="PSUM"))
                ps_o = tctx.enter_context(
                    tc.tile_pool(name=f"pso{rep}", bufs=2, space="PSUM"))
                ps_w = tctx.enter_context(
                    tc.tile_pool(name=f"psw{rep}", bufs=1, space="PSUM"))

                def rope_block(ps_nat, s0, dsts):
                    """RoPE a pair-packed psum block [128, ST] (2 heads x 64
                    rope dims); write f32 results to each (dst_ap, r0, r1)."""
                    tmp = rp_pool.tile([P, C.ST], F32, tag="rtmp")
                    nc.vector.tensor_copy(tmp[:], ps_nat[:])
                    qs = rp_pool.tile([P, C.ST], F32, tag="rqs")
                    for g in range(4):
                        nc.sync.dma_start(
                            out=qs[(g ^ 1) * 32:(g ^ 1) * 32 + 32, :],
                            in_=tmp[g * 32:(g + 1) * 32, :])
                    m1 = rp_pool.tile([P, C.ST], F32, tag="rm1")
                    nc.vector.tensor_mul(m1[:], tmp[:], cos_sb[:, s0:s0 + C.ST])
                    nc.vector.tensor_mul(qs[:], qs[:], ssin_sb[:, s0:s0 + C.ST])
                    for dst_ap, r0, r1 in dsts:
                        nc.vector.tensor_add(dst_ap, m1[r0:r1, :], qs[r0:r1, :])

                for st in range(NS):
                    s0 = st * C.ST
                    sl = slice(s0, s0 + C.ST)

                    # ===== Phase 1: q (fused LoRA) + kv latent from hidden =====
                    ht_sb = ht_tiles.pop(st)
                    if st + 1 < NS:
                        load_ht(st + 1)
                    kv_t = kv_pool.tile([P, KVC, C.ST], BF16)

                    def accum(lhs_sb, col0):
                        ps = psA.tile([P, C.ST], F32, tag="psA")
                        for h in range(HO):
                            nc.tensor.matmul(
                                ps[:], lhs_sb[h][:, col0:col0 + P],
                                ht_sb[:, h, :],
                                start=(h == 0), stop=(h == HO - 1))
                        return ps

                    for t in range(C.NH_G):
                        ps = accum(wqf_sb, t * P)
                        nc.vector.tensor_copy(qT_sb[:, t, 0, sl], ps[:])
                    for pr in range(NPAIR):
                        ps = accum(wqf_sb, C.QFN + pr * P)
                        h0, h1 = 2 * pr, 2 * pr + 1
                        rope_block(ps, s0, [
                            (qT_sb[0:64, h0, 1, sl], 0, 64),
                            (qT_sb[64:128, h1, 1, sl], 64, 128),
                        ])
                    for cc in range(KVC):
                        ps = accum(wkva_sb, cc * P)
                        nc.vector.tensor_copy(kv_t[:, cc, :], ps[:])

                    if st == 0:
                        nc.sync.dma_start(
                            out=wkn_sb[:],
                            in_=w_kbn.rearrange("(co ci) m -> ci co m", ci=P))
                        nc.sync.dma_start(
                            out=wkr_sb[:],
                            in_=w_kbr.rearrange("(co ci) m -> ci co m", ci=P))
                        nc.sync.dma_start(
                            out=wv_sb[:],
                            in_=w_vb.rearrange("(co ci) m -> ci co m", ci=P))

                    # ===== Phase 2: k / v head projections from kv latent =====
                    for h in range(C.NH_G):
                        ps = psA.tile([P, C.ST], F32, tag="psA")
                        for cc in range(KVC):
                            nc.tensor.matmul(
                                ps[:], wkn_sb[:, cc, h * C.DN:(h + 1) * C.DN],
                                kv_t[:, cc, :],
                                start=(cc == 0), stop=(cc == KVC - 1))
                        nc.vector.tensor_copy(kT_sb[:, h, 0, sl], ps[:])
                    for pr in range(NPAIR):
                        ps = psA.tile([P, C.ST], F32, tag="psA")
                        for cc in range(KVC):
                            nc.tensor.matmul(
                                ps[:], wkr_sb[:, cc, pr * P:(pr + 1) * P],
                                kv_t[:, cc, :],
                                start=(cc == 0), stop=(cc == KVC - 1))
                        h0, h1 = 2 * pr, 2 * pr + 1
                        rope_block(ps, s0, [(kT_sb[:, h0, 1, sl], 0, 128)])
                        nc.vector.tensor_copy(kT_sb[:, h1, 1, sl],
                                           kT_sb[:, h0, 1, sl])
                    for ssub in range(C.ST // P):
                        vs = (s0 + ssub * P) // P
                        ps = psA.tile([P, C.NH_G * DV], F32, tag="psA")
                        for cc in range(KVC):
                            nc.tensor.matmul(
                                ps[:], kv_t[:, cc, ssub * P:(ssub + 1) * P],
                                wv_sb[:, cc, :],
                                start=(cc == 0), stop=(cc == KVC - 1))
                        nc.vector.tensor_copy(v_sb[:, vs, :], ps[:])

                    if st == 0:
                        nc.sync.dma_start(
                            out=wo_sb[:],
                            in_=w_ob.rearrange("(h d) o -> d h o", d=P))
                        nc.sync.dma_start(
                            out=dm_sb[:],
                            in_=dmask.rearrange("(j ki) q -> ki j q", ki=P))

                    # ================= Phase 3: attention (qt = st) ============
                    qt = st
                    q0 = qt * C.QT
                    nkt = (qt + 1) * C.QT // P
                    ao_sb = ao_pool.tile([P, C.NH_G, C.QT], BF16, tag="ao")
                    for h in range(C.NH_G):
                        # two interleaved partial e-sums keep the DVE
                        # accumulation chains short
                        esum = [es_pool.tile([P, C.QT], BF16, tag="esA",
                                             name="esumA"),
                                es_pool.tile([P, C.QT], BF16, tag="esB",
                                             name="esumB")]
                        pso = ps_o.tile([P, C.QT], F32, tag="pso")
                        for kt in range(nkt):
                            j = kt - qt * NDIAG      # >= 0 -> diagonal tile
                            c0 = max(j, 0) * P       # first unmasked column
                            k0 = kt * P
                            pss = ps_s.tile([P, C.QT], F32, tag="pss")
                            nc.tensor.matmul(
                                pss[:, c0:], kT_sb[:, h, :, k0:k0 + P],
                                qT_sb[:, h, :, q0 + c0:q0 + C.QT],
                                start=True, stop=True,
                                perf_mode=mybir.MatmulPerfMode.DoubleRow)
                            e = e_pool.tile([P, C.QT], BF16, tag="e")
                            nc.scalar.activation(
                                e[:, c0:], pss[:, c0:],
                                mybir.ActivationFunctionType.Exp, scale=C.SCALE)
                            if j >= 0:
                                nc.vector.tensor_mul(
                                    e[:, c0:c0 + P], e[:, c0:c0 + P],
                                    dm_sb[:, j, c0:c0 + P])
                            es = esum[kt % 2]
                            if kt < 2:
                                if c0 > 0:
                                    nc.vector.memset(es[:, :c0], 0.0)
                                nc.vector.tensor_copy(es[:, c0:], e[:, c0:])
                            else:
                                nc.vector.tensor_add(
                                    es[:, c0:], es[:, c0:], e[:, c0:])
                            nc.tensor.matmul(
                                pso[:, c0:], v_sb[:, kt, h * DV:(h + 1) * DV],
                                e[:, c0:],
                                start=(kt == 0), stop=(kt == nkt - 1))
                        psd = ps_d.tile([P, C.QT], F32, tag="psd")
                        nterm = min(nkt, 2)
                        for i in range(nterm):
                            nc.tensor.matmul(psd[:], ones_sb[:], esum[i][:],
                                             start=(i == 0),
                                             stop=(i == nterm - 1))
                        rec = d_pool.tile([P, C.QT], F32, tag="rec")
                        nc.vector.reciprocal(rec[:], psd[:])
                        nc.vector.tensor_mul(ao_sb[:, h, :], pso[:], rec[:])
                    for qs in range(C.QT // P):
                        for ot in range(NOT):
                            psw = ps_w.tile([P, 512], F32, tag="psw")
                            for h in range(C.NH_G):
                                nc.tensor.matmul(
                                    psw[:], ao_sb[:, h, qs * P:(qs + 1) * P],
                                    wo_sb[:, h, ot * 512:(ot + 1) * 512],
                                    start=(h == 0), stop=(h == C.NH_G - 1))
                            oev = oev_pool.tile([P, 512], BF16)
                            nc.vector.tensor_copy(oev[:], psw[:])
                            nc.sync.dma_start(
                                out=outp[q0 + qs * P:q0 + (qs + 1) * P,
                                         ot * 512:(ot + 1) * 512],
                                in_=oev[:])

    nc.compile()
    return nc


def rope_tables(C: Cfg):
    """cos2/ssin2 [128, S] bf16: two stacked 64-row blocks (head pairs
    share); ssin has the rotate-half sign baked into the first 32 rows."""
    inv = 1.0 / (10000.0 ** (np.arange(0, C.DR, 2, dtype=np.float64) / C.DR))
    freqs = np.arange(C.S, dtype=np.float64)[:, None] * inv[None, :]  # [S, 32]
    emb = np.concatenate([freqs, freqs], axis=1)  # [S, 64]
    cos = np.cos(emb).T.astype(np.float32)   # [64, S]
    sin = np.sin(emb).T.astype(np.float32)
    ssin = sin.copy()
    ssin[: C.DR // 2] = -ssin[: C.DR // 2]
    cos2 = np.concatenate([cos, cos], axis=0)     # [128, S]
    ssin2 = np.concatenate([ssin, ssin], axis=0)
    bf = lambda x: np.ascontiguousarray(x).astype(ml_dtypes.bfloat16)
    return bf(cos2), bf(ssin2)


_FUSED_CACHE = {}


def _fused_wq(inputs):
    """w_q_a @ [w_q_nope | w_q_rope] for all heads, computed once."""
    key = id(inputs.get("w_q_a"))
    if _FUSED_CACHE.get("key") != key:
        w_q_a = np.asarray(inputs["w_q_a"], dtype=np.float32)
        wn = w_q_a @ np.asarray(inputs["w_q_nope"], dtype=np.float32)
        wr = w_q_a @ np.asarray(inputs["w_q_rope"], dtype=np.float32)
        _FUSED_CACHE.update(key=key, wn=wn, wr=wr)
    return _FUSED_CACHE["wn"], _FUSED_CACHE["wr"]


def host_inputs(C: Cfg, inputs: dict, core: int):
    """Build the per-core input map from full inputs."""
    NH = inputs["w_q_nope"].shape[1] // C.DN
    groups = NH // C.NH_G
    b = core // groups
    g = core % groups
    hs = slice(g * C.NH_G, (g + 1) * C.NH_G)

    bf = lambda x: np.ascontiguousarray(
        np.asarray(x, dtype=np.float32)).astype(ml_dtypes.bfloat16)

    wn_full, wr_full = _fused_wq(inputs)
    w_qfn = wn_full.reshape(C.HID, NH, C.DN)[:, hs].reshape(C.HID, -1)
    w_qfr = wr_full.reshape(C.HID, NH, C.DR)[:, hs].reshape(C.HID, -1)
    w_qf = bf(np.concatenate([w_qfn, w_qfr], axis=1))

    hT = bf(inputs["hidden_states"][b].T)
    w_kva = bf(inputs["w_kv_a"])
    w_kbn = bf(inputs["w_k_nope"].reshape(C.KVLR, NH, C.DN)[:, hs]
               .reshape(C.KVLR, -1))
    w_kbr = bf(inputs["w_k_rope"].reshape(C.KVLR, NH, C.DR)[:, hs]
               .reshape(C.KVLR, -1))
    w_vb = bf(inputs["w_v"].reshape(C.KVLR, NH, C.DV)[:, hs]
              .reshape(C.KVLR, -1))
    w_ob = bf(inputs["w_o"].reshape(NH, C.DV, C.HID)[hs].reshape(-1, C.HID))
    cos2, ssin2 = rope_tables(C)
    cm = np.asarray(inputs["causal_mask"])[0, 0]
    dmask = np.ascontiguousarray(
        cm[-C.QT:, -C.QT:].T.astype(np.float32)).astype(ml_dtypes.bfloat16)
    return {
        "hT": hT, "w_qf": w_qf, "w_kva": w_kva,
        "w_kbn": w_kbn, "w_kbr": w_kbr, "w_vb": w_vb, "w_ob": w_ob,
        "cos2": cos2, "ssin2": ssin2, "dmask": dmask,
    }


_NC_CACHE = {}


def kernel(**inputs) -> np.ndarray:
    from concourse.bass_utils import run_bass_kernel_spmd

    C = CFG
    if "nc" not in _NC_CACHE:
        _NC_CACHE["nc"] = build_nc(C)
    nc = _NC_CACHE["nc"]

    in_maps = [host_inputs(C, inputs, c) for c in range(8)]
    res = run_bass_kernel_spmd(nc, in_maps, core_ids=list(range(8)))

    B = inputs["hidden_states"].shape[0]
    groups = 8 // B
    out = np.zeros((B, C.S, C.HID), dtype=np.float32)
    for c in range(8):
        out[c // groups] += np.asarray(res.results[c]["outp"],
                                       dtype=np.float32)
    return out


# revision 15
# speedup vs baseline: 2.8010x; 1.6272x over previous
"""DeepseekV3 MLA attention kernel for 8 Trainium2 NeuronCores — v2.

Sharding: 2-way data-parallel over batch x 4-way tensor-parallel over heads.
Core c handles batch b = c // 4 and heads [4*(c%4) .. 4*(c%4)+4).

v2 design vs baseline:
  - all projection / AV / w_o matmuls in bf16 (same PE rate as fp32r but
    half the SBUF/DMA traffic, FWL weight loads, 2x DVE elementwise)
  - score matmuls in fp8e4 with DoubleRow perf mode: nope(128) + rope(64,
    zero-padded) packed as a 2-plane 256-deep contraction -> one matmul
    at 0.5 cycles/row
  - softmax denominators via DVE running-sum of e-tiles + one ones-matmul
    per (head, q-tile) instead of a ones-matmul per (head, k-tile)
  - causal diag shrink: score/exp/AV restricted to the unmasked column
    range on diagonal tiles
  - all intermediates (q, k, v) SBUF-resident, no DRAM scratch
  - phases software-pipelined per 512-wide s-tile:
    p1(st) -> p2(st) -> attention(qt=st)

All tolerances validated numerically on the CPU reference: bf16-everything
gives 4.3e-3 scale-relative max error; fp8 q/k adds ~6.7e-3 (tolerance 2e-2).
"""

from contextlib import ExitStack
from dataclasses import dataclass

import numpy as np
import ml_dtypes

import concourse.bacc as bacc
import concourse.mybir as mybir
import concourse.tile as tile

F32 = mybir.dt.float32
BF16 = mybir.dt.bfloat16
FP8 = mybir.dt.float8e4


@dataclass(frozen=True)
class Cfg:
    S: int = 2048          # sequence length (per batch)
    HID: int = 2048        # hidden dim
    QLR: int = 1536        # q lora rank (host-side only)
    KVLR: int = 512        # kv lora rank
    NH_G: int = 4          # heads per core
    DN: int = 128          # nope dim
    DR: int = 64           # rope dim
    DV: int = 128          # v head dim
    ST: int = 512          # phase-1/2 s-tile width
    QT: int = 512          # attention q-tile width

    @property
    def QFN(self):
        return self.NH_G * self.DN      # fused q nope cols (512)

    @property
    def QFR(self):
        return (self.NH_G // 2) * 128   # fused q rope cols, pair-packed (256)

    @property
    def SCALE(self):
        return 1.0 / float(np.sqrt(self.DN + self.DR))


CFG = Cfg()


def build_nc(C: Cfg, reps: int = 1):
    nc = bacc.Bacc("TRN2", target_bir_lowering=False, debug=False, num_devices=8)
    P = 128
    HO = C.HID // P          # 16
    NS = C.S // C.ST         # 4
    KVC = C.KVLR // P        # 4
    NPAIR = C.NH_G // 2      # 2
    NQT = C.S // C.QT        # 4
    NDIAG = C.QT // P        # 4
    NVS = C.S // P           # 16
    NOT = C.HID // 512       # 4
    DV = C.DV

    # ---- kernel I/O ----
    hT = nc.dram_tensor("hT", [C.HID, C.S], BF16, kind="ExternalInput").ap()
    w_qf = nc.dram_tensor("w_qf", [C.HID, C.QFN + C.QFR], BF16,
                          kind="ExternalInput").ap()
    w_kva = nc.dram_tensor("w_kva", [C.HID, C.KVLR], BF16,
                           kind="ExternalInput").ap()
    w_kbn = nc.dram_tensor("w_kbn", [C.KVLR, C.QFN], BF16,
                           kind="ExternalInput").ap()
    w_kbr = nc.dram_tensor("w_kbr", [C.KVLR, C.QFR], BF16,
                           kind="ExternalInput").ap()
    w_vb = nc.dram_tensor("w_vb", [C.KVLR, C.NH_G * DV], BF16,
                          kind="ExternalInput").ap()
    w_ob = nc.dram_tensor("w_ob", [C.NH_G * DV, C.HID], BF16,
                          kind="ExternalInput").ap()
    cos2 = nc.dram_tensor("cos2", [P, C.S], BF16, kind="ExternalInput").ap()
    ssin2 = nc.dram_tensor("ssin2", [P, C.S], BF16, kind="ExternalInput").ap()
    dmask = nc.dram_tensor("dmask", [C.QT, C.QT], BF16, kind="ExternalInput").ap()
    outp = nc.dram_tensor("outp", [C.S, C.HID], BF16, kind="ExternalOutput").ap()

    hT_r = hT.rearrange("(ho hi) s -> hi ho s", hi=P)

    with tile.TileContext(nc) as tc:
        with ExitStack() as tctx:
            per = tctx.enter_context(tc.tile_pool(name="per", bufs=1))
            ht_pool = tctx.enter_context(tc.tile_pool(name="ht", bufs=2))
            kv_pool = tctx.enter_context(tc.tile_pool(name="kv", bufs=2))
            rp_pool = tctx.enter_context(tc.tile_pool(name="rp", bufs=2))
            e_pool = tctx.enter_context(tc.tile_pool(name="e", bufs=6))
            es_pool = tctx.enter_context(tc.tile_pool(name="es", bufs=2))
            d_pool = tctx.enter_context(tc.tile_pool(name="d", bufs=2))
            ao_pool = tctx.enter_context(tc.tile_pool(name="ao", bufs=2))
            oev_pool = tctx.enter_context(tc.tile_pool(name="oe", bufs=2))
            psA = tctx.enter_context(
                tc.tile_pool(name="psA", bufs=2, space="PSUM"))
            ps_s = tctx.enter_context(
                tc.tile_pool(name="pss", bufs=2, space="PSUM"))
            ps_d = tctx.enter_context(
                tc.tile_pool(name="psd", bufs=1, space="PSUM"))
            ps_o = tctx.enter_context(
                tc.tile_pool(name="pso", bufs=2, space="PSUM"))
            ps_w = tctx.enter_context(
                tc.tile_pool(name="psw", bufs=1, space="PSUM"))
            for rep in range(reps):
                # persistent tiles
                cos_sb = per.tile([P, C.S], BF16)
                ssin_sb = per.tile([P, C.S], BF16)
                dm_sb = per.tile([P, NDIAG, C.QT], BF16)
                ones_sb = per.tile([P, P], BF16)
                wqf_sb = [per.tile([P, C.QFN + C.QFR], BF16, tag=f"wqf{ho}",
                                   name=f"wqf{ho}") for ho in range(HO)]
                wkva_sb = [per.tile([P, C.KVLR], BF16, tag=f"wkva{ho}",
                                    name=f"wkva{ho}") for ho in range(HO)]
                wkn_sb = per.tile([P, KVC, C.QFN], BF16)
                wkr_sb = per.tile([P, KVC, C.QFR], BF16)
                wv_sb = per.tile([P, KVC, C.NH_G * DV], BF16)
                wo_sb = per.tile([P, C.NH_G, C.HID], BF16)
                # q/k in fp8, 2 planes: 0 = nope, 1 = rope (pair-packed)
                qT_sb = per.tile([P, C.NH_G, 2, C.S], FP8)
                kT_sb = per.tile([P, C.NH_G, 2, C.S], FP8)
                v_sb = per.tile([P, NVS, C.NH_G * DV], BF16)

                # hT prefetch: st=0 queued before any weight DMA so the
                # first accumulation can start immediately
                ht_tiles = {}

                def load_ht(st):
                    t = ht_pool.tile([P, HO, C.ST], BF16, tag="ht")
                    s = slice(st * C.ST, (st + 1) * C.ST)
                    for ho in range(0, HO, 2):
                        nc.sync.dma_start(
                            out=t[:, ho:ho + 2, :], in_=hT_r[:, ho:ho + 2, s])
                    ht_tiles[st] = t

                load_ht(0)
                wqf_r = w_qf.rearrange("(ho hi) c -> hi ho c", hi=P)
                wkva_r = w_kva.rearrange("(ho hi) c -> hi ho c", hi=P)
                for ho in range(HO):
                    nc.sync.dma_start(out=wqf_sb[ho][:], in_=wqf_r[:, ho, :])
                for ho in range(HO):
                    nc.sync.dma_start(out=wkva_sb[ho][:], in_=wkva_r[:, ho, :])
                nc.sync.dma_start(out=cos_sb[:], in_=cos2)
                nc.sync.dma_start(out=ssin_sb[:], in_=ssin2)
                nc.gpsimd.memset(ones_sb[:], 1.0)
                # zero the unused rope half of each head's q plane 1 (the k
                # plane 1 keeps the full head pair; the q-side zeros select
                # this head's rope rows in the DoubleRow contraction)
                for h in range(C.NH_G):
                    if h % 2 == 0:
                        nc.gpsimd.memset(qT_sb[64:128, h, 1, :], 0.0)
                    else:
                        nc.gpsimd.memset(qT_sb[0:64, h, 1, :], 0.0)
                def rope_block(ps_nat, s0, dsts):
                    """RoPE a pair-packed psum block [128, ST] (2 heads x 64
                    rope dims); write f32 results to each (dst_ap, r0, r1)."""
                    tmp = rp_pool.tile([P, C.ST], F32, tag="rtmp")
                    nc.vector.tensor_copy(tmp[:], ps_nat[:])
                    qs = rp_pool.tile([P, C.ST], F32, tag="rqs")
                    for g in range(4):
                        nc.sync.dma_start(
                            out=qs[(g ^ 1) * 32:(g ^ 1) * 32 + 32, :],
                            in_=tmp[g * 32:(g + 1) * 32, :])
                    m1 = rp_pool.tile([P, C.ST], F32, tag="rm1")
                    nc.vector.tensor_mul(m1[:], tmp[:], cos_sb[:, s0:s0 + C.ST])
                    nc.vector.tensor_mul(qs[:], qs[:], ssin_sb[:, s0:s0 + C.ST])
                    for dst_ap, r0, r1 in dsts:
                        nc.vector.tensor_add(dst_ap, m1[r0:r1, :], qs[r0:r1, :])

                for st in range(NS):
                    s0 = st * C.ST
                    sl = slice(s0, s0 + C.ST)

                    # ===== Phase 1: q (fused LoRA) + kv latent from hidden =====
                    ht_sb = ht_tiles.pop(st)
                    if st + 1 < NS:
                        load_ht(st + 1)
                    kv_t = kv_pool.tile([P, KVC, C.ST], BF16)

                    def accum(lhs_sb, col0):
                        ps = psA.tile([P, C.ST], F32, tag="psA")
                        for h in range(HO):
                            nc.tensor.matmul(
                                ps[:], lhs_sb[h][:, col0:col0 + P],
                                ht_sb[:, h, :],
                                start=(h == 0), stop=(h == HO - 1))
                        return ps

                    for t in range(C.NH_G):
                        ps = accum(wqf_sb, t * P)
                        nc.vector.tensor_copy(qT_sb[:, t, 0, sl], ps[:])
                    for pr in range(NPAIR):
                        ps = accum(wqf_sb, C.QFN + pr * P)
                        h0, h1 = 2 * pr, 2 * pr + 1
                        rope_block(ps, s0, [
                            (qT_sb[0:64, h0, 1, sl], 0, 64),
                            (qT_sb[64:128, h1, 1, sl], 64, 128),
                        ])
                    for cc in range(KVC):
                        ps = accum(wkva_sb, cc * P)
                        nc.vector.tensor_copy(kv_t[:, cc, :], ps[:])

                    if st == 0:
                        nc.sync.dma_start(
                            out=wkn_sb[:],
                            in_=w_kbn.rearrange("(co ci) m -> ci co m", ci=P))
                        nc.sync.dma_start(
                            out=wkr_sb[:],
                            in_=w_kbr.rearrange("(co ci) m -> ci co m", ci=P))
                        nc.sync.dma_start(
                            out=wv_sb[:],
                            in_=w_vb.rearrange("(co ci) m -> ci co m", ci=P))

                    # ===== Phase 2: k / v head projections from kv latent =====
                    for h in range(C.NH_G):
                        ps = psA.tile([P, C.ST], F32, tag="psA")
                        for cc in range(KVC):
                            nc.tensor.matmul(
                                ps[:], wkn_sb[:, cc, h * C.DN:(h + 1) * C.DN],
                                kv_t[:, cc, :],
                                start=(cc == 0), stop=(cc == KVC - 1))
                        nc.vector.tensor_copy(kT_sb[:, h, 0, sl], ps[:])
                    for pr in range(NPAIR):
                        ps = psA.tile([P, C.ST], F32, tag="psA")
                        for cc in range(KVC):
                            nc.tensor.matmul(
                                ps[:], wkr_sb[:, cc, pr * P:(pr + 1) * P],
                                kv_t[:, cc, :],
                                start=(cc == 0), stop=(cc == KVC - 1))
                        h0, h1 = 2 * pr, 2 * pr + 1
                        rope_block(ps, s0, [(kT_sb[:, h0, 1, sl], 0, 128)])
                        nc.vector.tensor_copy(kT_sb[:, h1, 1, sl],
                                           kT_sb[:, h0, 1, sl])
                    for ssub in range(C.ST // P):
                        vs = (s0 + ssub * P) // P
                        ps = psA.tile([P, C.NH_G * DV], F32, tag="psA")
                        for cc in range(KVC):
                            nc.tensor.matmul(
                                ps[:], kv_t[:, cc, ssub * P:(ssub + 1) * P],
                                wv_sb[:, cc, :],
                                start=(cc == 0), stop=(cc == KVC - 1))
                        nc.vector.tensor_copy(v_sb[:, vs, :], ps[:])

                    if st == 0:
                        nc.sync.dma_start(
                            out=wo_sb[:],
                            in_=w_ob.rearrange("(h d) o -> d h o", d=P))
                        nc.sync.dma_start(
                            out=dm_sb[:],
                            in_=dmask.rearrange("(j ki) q -> ki j q", ki=P))

                    # ================= Phase 3: attention (qt = st) ============
                    qt = st
                    q0 = qt * C.QT
                    nkt = (qt + 1) * C.QT // P
                    ao_sb = ao_pool.tile([P, C.NH_G, C.QT], BF16, tag="ao")
                    for h in range(C.NH_G):
                        # two interleaved partial e-sums keep the DVE
                        # accumulation chains short
                        esum = [es_pool.tile([P, C.QT], BF16, tag="esA",
                                             name="esumA"),
                                es_pool.tile([P, C.QT], BF16, tag="esB",
                                             name="esumB")]
                        pso = ps_o.tile([P, C.QT], F32, tag="pso")
                        for kt in range(nkt):
                            j = kt - qt * NDIAG      # >= 0 -> diagonal tile
                            c0 = max(j, 0) * P       # first unmasked column
                            k0 = kt * P
                            pss = ps_s.tile([P, C.QT], F32, tag="pss")
                            nc.tensor.matmul(
                                pss[:, c0:], kT_sb[:, h, :, k0:k0 + P],
                                qT_sb[:, h, :, q0 + c0:q0 + C.QT],
                                start=True, stop=True,
                                perf_mode=mybir.MatmulPerfMode.DoubleRow)
                            e = e_pool.tile([P, C.QT], BF16, tag="e")
                            nc.scalar.activation(
                                e[:, c0:], pss[:, c0:],
                                mybir.ActivationFunctionType.Exp, scale=C.SCALE)
                            if j >= 0:
                                nc.vector.tensor_mul(
                                    e[:, c0:c0 + P], e[:, c0:c0 + P],
                                    dm_sb[:, j, c0:c0 + P])
                            es = esum[kt % 2]
                            if kt < 2:
                                if c0 > 0:
                                    nc.vector.memset(es[:, :c0], 0.0)
                                nc.vector.tensor_copy(es[:, c0:], e[:, c0:])
                            else:
                                nc.vector.tensor_add(
                                    es[:, c0:], es[:, c0:], e[:, c0:])
                            nc.tensor.matmul(
                                pso[:, c0:], v_sb[:, kt, h * DV:(h + 1) * DV],
                                e[:, c0:],
                                start=(kt == 0), stop=(kt == nkt - 1))
                        psd = ps_d.tile([P, C.QT], F32, tag="psd")
                        nterm = min(nkt, 2)
                        for i in range(nterm):
                            nc.tensor.matmul(psd[:], ones_sb[:], esum[i][:],
                                             start=(i == 0),
                                             stop=(i == nterm - 1))
                        rec = d_pool.tile([P, C.QT], F32, tag="rec")
                        nc.vector.reciprocal(rec[:], psd[:])
                        nc.vector.tensor_mul(ao_sb[:, h, :], pso[:], rec[:])
                    for qs in range(C.QT // P):
                        for ot in range(NOT):
                            psw = ps_w.tile([P, 512], F32, tag="psw")
                            for h in range(C.NH_G):
                                nc.tensor.matmul(
                                    psw[:], ao_sb[:, h, qs * P:(qs + 1) * P],
                                    wo_sb[:, h, ot * 512:(ot + 1) * 512],
                                    start=(h == 0), stop=(h == C.NH_G - 1))
                            oev = oev_pool.tile([P, 512], BF16)
                            nc.vector.tensor_copy(oev[:], psw[:])
                            nc.sync.dma_start(
                                out=outp[q0 + qs * P:q0 + (qs + 1) * P,
                                         ot * 512:(ot + 1) * 512],
                                in_=oev[:])

    nc.compile()
    return nc


def rope_tables(C: Cfg):
    """cos2/ssin2 [128, S] bf16: two stacked 64-row blocks (head pairs
    share); ssin has the rotate-half sign baked into the first 32 rows."""
    inv = 1.0 / (10000.0 ** (np.arange(0, C.DR, 2, dtype=np.float64) / C.DR))
    freqs = np.arange(C.S, dtype=np.float64)[:, None] * inv[None, :]  # [S, 32]
    emb = np.concatenate([freqs, freqs], axis=1)  # [S, 64]
    cos = np.cos(emb).T.astype(np.float32)   # [64, S]
    sin = np.sin(emb).T.astype(np.float32)
    ssin = sin.copy()
    ssin[: C.DR // 2] = -ssin[: C.DR // 2]
    cos2 = np.concatenate([cos, cos], axis=0)     # [128, S]
    ssin2 = np.concatenate([ssin, ssin], axis=0)
    bf = lambda x: np.ascontiguousarray(x).astype(ml_dtypes.bfloat16)
    return bf(cos2), bf(ssin2)


_FUSED_CACHE = {}


def _fused_wq(inputs):
    """w_q_a @ [w_q_nope | w_q_rope] for all heads, computed once."""
    key = id(inputs.get("w_q_a"))
    if _FUSED_CACHE.get("key") != key:
        w_q_a = np.asarray(inputs["w_q_a"], dtype=np.float32)
        wn = w_q_a @ np.asarray(inputs["w_q_nope"], dtype=np.float32)
        wr = w_q_a @ np.asarray(inputs["w_q_rope"], dtype=np.float32)
        _FUSED_CACHE.update(key=key, wn=wn, wr=wr)
    return _FUSED_CACHE["wn"], _FUSED_CACHE["wr"]


def host_inputs(C: Cfg, inputs: dict, core: int):
    """Build the per-core input map from full inputs."""
    NH = inputs["w_q_nope"].shape[1] // C.DN
    groups = NH // C.NH_G
    b = core // groups
    g = core % groups
    hs = slice(g * C.NH_G, (g + 1) * C.NH_G)

    bf = lambda x: np.ascontiguousarray(
        np.asarray(x, dtype=np.float32)).astype(ml_dtypes.bfloat16)

    wn_full, wr_full = _fused_wq(inputs)
    w_qfn = wn_full.reshape(C.HID, NH, C.DN)[:, hs].reshape(C.HID, -1)
    w_qfr = wr_full.reshape(C.HID, NH, C.DR)[:, hs].reshape(C.HID, -1)
    w_qf = bf(np.concatenate([w_qfn, w_qfr], axis=1))

    hT = bf(inputs["hidden_states"][b].T)
    w_kva = bf(inputs["w_kv_a"])
    w_kbn = bf(inputs["w_k_nope"].reshape(C.KVLR, NH, C.DN)[:, hs]
               .reshape(C.KVLR, -1))
    w_kbr = bf(inputs["w_k_rope"].reshape(C.KVLR, NH, C.DR)[:, hs]
               .reshape(C.KVLR, -1))
    w_vb = bf(inputs["w_v"].reshape(C.KVLR, NH, C.DV)[:, hs]
              .reshape(C.KVLR, -1))
    w_ob = bf(inputs["w_o"].reshape(NH, C.DV, C.HID)[hs].reshape(-1, C.HID))
    cos2, ssin2 = rope_tables(C)
    cm = np.asarray(inputs["causal_mask"])[0, 0]
    dmask = np.ascontiguousarray(
        cm[-C.QT:, -C.QT:].T.astype(np.float32)).astype(ml_dtypes.bfloat16)
    return {
        "hT": hT, "w_qf": w_qf, "w_kva": w_kva,
        "w_kbn": w_kbn, "w_kbr": w_kbr, "w_vb": w_vb, "w_ob": w_ob,
        "cos2": cos2, "ssin2": ssin2, "dmask": dmask,
    }


_NC_CACHE = {}


def kernel(**inputs) -> np.ndarray:
    from concourse.bass_utils import run_bass_kernel_spmd

    C = CFG
    if "nc" not in _NC_CACHE:
        _NC_CACHE["nc"] = build_nc(C)
    nc = _NC_CACHE["nc"]

    in_maps = [host_inputs(C, inputs, c) for c in range(8)]
    res = run_bass_kernel_spmd(nc, in_maps, core_ids=list(range(8)))

    B = inputs["hidden_states"].shape[0]
    groups = 8 // B
    out = np.zeros((B, C.S, C.HID), dtype=np.float32)
    for c in range(8):
        out[c // groups] += np.asarray(res.results[c]["outp"],
                                       dtype=np.float32)
    return out
